# revision 91
# baseline (speedup 1.0000x reference)
"""MiMoV2 attention (GQA + partial RoPE + attention sinks + causal) on 8 TRN2
NeuronCores.

Sharding: tensor-parallel over heads. Core c owns KV head c and query heads
[4c, 4c+4). Wq/Wk/Wv split along output dim, Wo along input dim; each core
computes a partial output [S, H] which the host sums (the Wo contraction over
heads distributes over cores).

Per-core dataflow (everything head-transposed so no on-chip transposes needed):
  hsT [H, S] streamed by 128-row h-tiles; per s-chunk of 512:
    QT[d, s] (4 heads), KT[d, s] accumulate in PSUM over the h-tiles
    V[s, d] natural layout via hsT-as-stationary matmuls

  The QKV and output projections run as error-compensated fp8 matmuls in
  DoubleRow perf mode: x ~= x_hi + x_lo (both f8e4m3, residual split), and
  x@w = xh@wh + (xh@wl + xl@wh), dropping the lo*lo term (~0.07% error,
  below bf16 noise). The hi@hi pass packs two 128-deep k-tiles per DoubleRow
  instruction; the cross pass packs both cross products of one k-tile per
  instruction (w stored (hi,lo), x stored (lo,hi)). 3 passes over 2x-rate
  fp8 = 0.75x the PE cycles of bf16 at ~equal accuracy. Weights are
  pre-scaled by 512 on the host (fp8 subnormal floor), descaled in the
  PSUM->SBUF copy-outs (rope cos/sin pre-scaled, ACT copy-with-scale
  elsewhere).

  partial RoPE applied in [d, s] layout; rotate_half reads the PSUM rows
  cross-partition with the sign folded into sinT. V transposes to [s, d]
  via PE transpose matmuls (HWDGE generation is a serial resource the hst
  stream needs). scoresT[s_k, s_q] = KT_tile^T @ QT-chunk (bf16: with a
  128-deep contraction, 3-pass fp8 DoubleRow would cost 1.5x, so bf16 is
  optimal here); exp on ACT (no max subtraction -- |scores| <= ~12);
  causal via binary mask multiply on diagonal tiles only. attn_outT[d,
  s_q] accumulates V_tile^T @ probsT (bf16). The softmax denominator
  accumulates the probs tiles on DVE and Pool (2:1, matching their
  throughputs) seeded with exp(sink)/128, and finishes with two all-ones
  matmuls per (chunk, head) -- vs one per score tile -- freeing ~8% of PE
  cycles; division by DVE reciprocal + multiply, then the attnT chunk is
  split hi/lo fp8 (Pool; DVE for the head gating the final phase).
  out_partial[s, o] = attnT-as-stationary @ Wo-chunk (fp8 DoubleRow over
  the 4 local heads), interleaved tile-by-tile into the exp-paced
  attention chunks; written out as bf16 in 1024-col strips; host sums the
  8 partials in fp32.

  Emission order is the PE schedule: p1(0), p2(0), p1(1), p1(2),
  p2(1)+p3(0), p1(3), p2(2)+p3(1), p2(3)+p3(2), p3(3), with p3 s-tiles
  half-lagged behind the heads so the last head's softmax tail overlaps
  projection matmuls.
"""

import numpy as np
import ml_dtypes
from contextlib import ExitStack

import concourse.bass as bass
import concourse.mybir as mybir
import concourse.tile as tile
from concourse.bass_utils import run_bass_kernel_spmd

bf16 = ml_dtypes.bfloat16
f8 = ml_dtypes.float8_e4m3
BF = mybir.dt.bfloat16
F32 = mybir.dt.float32
F8 = mybir.dt.float8e4
DR = mybir.MatmulPerfMode.DoubleRow

N_CORES = 8
S = 2048
H = 4096
HD = 128
ROPE = 64
NHL = 4                    # local query heads per core
CH = 512                   # s-chunk width
NCHUNK = S // CH           # 4
HT = H // 128              # 32 h-tiles
NKT = S // 128             # 16 k-tiles
SW = 512.0                 # host-side fp8 weight pre-scale (power of 2)
DESC = 1.0 / SW

# this walrus build allows at most one sync wait per instruction
_MAX_WAITS = 1


def _split_excess_waits(nc):
    cnt = 0
    for f in nc.m.functions:
        for bb in f.blocks:
            out, changed = [], False
            for inst in bb.instructions:
                si = inst.sync_info
                if si is not None and len(si.on_wait) > _MAX_WAITS:
                    waits = list(si.on_wait)
                    excess, keep = waits[:-_MAX_WAITS], waits[-_MAX_WAITS:]
                    for i in range(0, len(excess), _MAX_WAITS):
                        cnt += 1
                        out.append(mybir.InstNoOp(
                            name=f"waitnop-{cnt}", engine=inst.engine,
                            sync_info=mybir.SyncInfo(
                                on_wait=excess[i:i + _MAX_WAITS], on_update=[])))
                    si.on_wait = keep
                    changed = True
                out.append(inst)
            if changed:
                bb.instructions = out
    return cnt


def _rope_copy(nc, pool, psum_t, dest, cos_sb, sin_sb, sl):
    """psum_t [128,512] fp32 -> dest [128,512] bf16 slice, applying partial
    RoPE to rows 0:64 (rotate_half = +-32-partition swap done by reading the
    PSUM rows cross-partition, sign pre-folded into sin_sb). cos/sin are
    pre-scaled by DESC on the host, and the pass-through copy descales on
    ACT, so the fp8 weight scale cancels here.
    """
    # pass-through rows 64:128 on ACT (keeps DVE free), descaled
    nc.scalar.mul(dest[64:128, :], psum_t[64:128, :], DESC)
    # t2 = rotate_half(q_r) * sin, reading the swapped halves straight from
    # PSUM (cross-partition operand offsets)
    t2 = pool.tile([64, CH], BF, tag="rope_t2")
    nc.vector.tensor_mul(t2[0:32, :], psum_t[32:64, :], sin_sb[0:32, sl])
    nc.vector.tensor_mul(t2[32:64, :], psum_t[0:32, :], sin_sb[32:64, sl])
    # t1 = q_r * cos   (one fused op: (psum mult 1.0) mult cos)
    t1 = pool.tile([64, CH], BF, tag="rope_t1")
    nc.vector.scalar_tensor_tensor(
        t1[:, :], psum_t[0:64, :], 1.0, cos_sb[:, sl],
        op0=mybir.AluOpType.mult, op1=mybir.AluOpType.mult)
    nc.vector.tensor_add(dest[0:64, :], t1[:, :], t2[:, :])


def build_bass(repeat=1):
    """repeat>1 duplicates the whole compute body (for timing: the wall-clock
    delta between repeat=2 and repeat=1 NEFFs is one kernel iteration,
    independent of the large fixed PJRT/axon dispatch overhead)."""
    nc = bass.Bass("TRN2", target_bir_lowering=False, debug=False)

    # fp8 hi/lo-split tensors: `two` holds (lo, hi) for moving/rhs-side
    # tensors and (hi, lo) for stationary/lhsT-side tensors so one DoubleRow
    # instruction covers both cross terms. hs8 rows are (ktile, partition)
    # and columns (chunk, two, s) so one 3-dim DMA moves a whole
    # [4-ktile, both-plane, chunk] tile.
    hs8 = nc.dram_tensor("hs8", [HT * 128, 2 * S], F8, kind="ExternalInput")
    wq8 = nc.dram_tensor("wq8", [HT * 128 * 2, NHL * HD], F8, kind="ExternalInput")
    wk8 = nc.dram_tensor("wk8", [HT * 128 * 2, HD], F8, kind="ExternalInput")
    wv8 = nc.dram_tensor("wv8", [HT * 128 * 2, HD], F8, kind="ExternalInput")
    wo8 = nc.dram_tensor("wo8", [NHL * 128 * 2, H], F8, kind="ExternalInput")
    cosT = nc.dram_tensor("cosT", [ROPE, S], BF, kind="ExternalInput")
    sinTs = nc.dram_tensor("sinTs", [ROPE, S], BF, kind="ExternalInput")
    esink = nc.dram_tensor("esink", [128, NHL], F32, kind="ExternalInput")
    ident = nc.dram_tensor("ident", [128, 128], BF, kind="ExternalInput")
    maskb = nc.dram_tensor("maskb", [128, 1024], BF, kind="ExternalInput")
    outp = nc.dram_tensor("outp", [S, H], BF, kind="ExternalOutput")

    with tile.TileContext(nc) as tc, ExitStack() as ctx:
        const = ctx.enter_context(tc.tile_pool(name="const", bufs=1))
        hs_pool = ctx.enter_context(tc.tile_pool(name="hs", bufs=9))
        rope_pool = ctx.enter_context(tc.tile_pool(name="rope", bufs=2))
        probs_pool = ctx.enter_context(tc.tile_pool(name="probs", bufs=9))
        den_pool = ctx.enter_context(tc.tile_pool(name="den", bufs=2))
        dacc_pool = ctx.enter_context(tc.tile_pool(name="dacc", bufs=2))
        att_pool = ctx.enter_context(tc.tile_pool(name="att", bufs=3))
        out_pool = ctx.enter_context(tc.tile_pool(name="out", bufs=4))

        # ---- constants / weights resident in SBUF ----
        # weights are loaded in h-tile groups so the first projection matmuls
        # only wait on the first slice, not the whole tensor
        wq_sb = const.tile([128, HT, 2, NHL * HD], F8)
        wk_sb = const.tile([128, HT, 2, HD], F8)
        wv_sb = const.tile([128, HT, 2, HD], F8)
        wq_r = wq8.rearrange("(t p two) c -> p t two c", p=128, two=2)
        wk_r = wk8.rearrange("(t p two) c -> p t two c", p=128, two=2)
        wv_r = wv8.rearrange("(t p two) c -> p t two c", p=128, two=2)
        hsT_r = hs8.rearrange("(t p) (c x) -> p t c x", p=128, c=NCHUNK)
        wo_sb = const.tile([128, NHL, 2, H], F8)
        wo_r = wo8.rearrange("(t p two) c -> p t two c", p=128, two=2)
        cos_sb = const.tile([ROPE, S], BF)
        sin_sb = const.tile([ROPE, S], BF)
        mask_sb = const.tile([128, 1024], BF)
        esink_sb = const.tile([128, NHL], F32)
        nc.gpsimd.dma_start(out=esink_sb, in_=esink[:, :])
        ident_sb = const.tile([128, 128], BF)
        nc.gpsimd.dma_start(out=ident_sb, in_=ident[:, :])
        ones_sb = const.tile([128, 128], BF)
        nc.vector.memset(ones_sb[:, :], 1.0)

        # persistent activations
        qt_sb = const.tile([128, NHL, S], BF)     # QT per head [d, s]
        kt_sb = const.tile([128, S], BF)          # KT [d, s]
        vt_sb = const.tile([128, S], BF)          # VT [d, s] (pre-transpose)
        v_sb = const.tile([128, NKT, HD], BF)     # V [s(128), kt, d]
        # attnT (hi,lo) [d, s], split by head PAIR: coarse per-tile write
        # tracking otherwise makes every p3 matmul wait on the latest head's
        # split (the hi@hi insts need each pair adjacent, so no finer split)
        at8a = const.tile([128, 2, 2, S], F8)
        at8b = const.tile([128, 2, 2, S], F8)

        for _rep in range(repeat):
            # phases 1+2 share one PSUM scope (8 banks: proj 3 + ps 2 + po 2
            # + pd 1) so projection chunks and attention chunks interleave on
            # PE with no pool-boundary serialization.
            with ExitStack() as p12:
                proj_pool = p12.enter_context(
                    tc.tile_pool(name="proj", bufs=3, space="PSUM"))
                ps_pool = p12.enter_context(
                    tc.tile_pool(name="ps", bufs=2, space="PSUM"))
                po_pool = p12.enter_context(
                    tc.tile_pool(name="po", bufs=2, space="PSUM"))
                ptp_pool = p12.enter_context(
                    tc.tile_pool(name="ptp", bufs=1, space="PSUM"))

                def emit_p1(ci, load_weights=False):
                    """QKV projections + RoPE for s-chunk ci. Each group's 48
                    DoubleRow matmuls (16 hi@hi ktile-pairs + 32 cross) form
                    one contiguous PSUM accumulation group in a single bank."""
                    sl = bass.ds(ci * CH, CH)
                    hs4 = []
                    for g4 in range(HT // 4):
                        h4 = hs_pool.tile([128, 4, 2, CH], F8, tag="hst",
                                          name=f"hst_{_rep}_{ci}_{g4}")
                        g = g4 * 4
                        # DMA APs allow max 3 dims: the s-chunk slice blocks
                        # (ktile,two) merging, so move each fp8 plane
                        # separately. The hst stream owns the SP queue; the
                        # weights go on the Pool queue so neither blocks the
                        # other at the queue head.
                        if load_weights and g4 % 2 == 0:
                            # interleave weight-slice loads with the hst
                            # stream (k first: the first matmuls are group
                            # k's) so the first matmuls start early
                            nc.sync.dma_start(out=wk_sb[:, g:g + 8],
                                              in_=wk_r[:, g:g + 8])
                        nc.sync.dma_start(out=h4, in_=hsT_r[:, g:g + 4, ci, :])
                        if load_weights and g4 % 2 == 0:
                            if g4 == 6:
                                # the tail of the stream is latency-critical:
                                # split the last wq batch so block 6 doesn't
                                # wait on the full 1MB transfer
                                nc.sync.dma_start(out=wq_sb[:, 24:28],
                                                  in_=wq_r[:, 24:28])
                                nc.sync.dma_start(out=wq_sb[:, 28:32],
                                                  in_=wq_r[:, 28:32])
                            else:
                                nc.sync.dma_start(out=wq_sb[:, g:g + 8],
                                                  in_=wq_r[:, g:g + 8])
                        if load_weights:
                            if g4 == 5:
                                # rope/mask constants: queued behind the
                                # critical weight stream, ready well
                                # before the first rope copy-out
                                nc.sync.dma_start(out=cos_sb,
                                                  in_=cosT[:, :])
                                nc.sync.dma_start(out=sin_sb,
                                                  in_=sinTs[:, :])
                                nc.sync.dma_start(out=mask_sb,
                                                  in_=maskb[:, :])
                        hs4.append(h4)
                    if load_weights:
                        # wv rides at the end of the stream: the v group's
                        # matmuls are deferred past the interleave, so its
                        # 1MB stays off the critical early window
                        nc.sync.dma_start(out=wv_sb[:, 0:16],
                                          in_=wv_r[:, 0:16])
                        nc.sync.dma_start(out=wv_sb[:, 16:32],
                                          in_=wv_r[:, 16:32])

                    def copy_out(pp, rope):
                        if rope is not None:
                            _rope_copy(nc, rope_pool, pp, rope, cos_sb, sin_sb, sl)
                        else:
                            nc.vector.tensor_scalar_mul(vt_sb[:, sl], pp[:, :],
                                                        DESC)
                            # transpose to [s, d] on PE (53ns/tile) instead of
                            # 4 DMAs: HWDGE generation is a serial resource
                            # the hst stream needs
                            ptp = ptp_pool.tile([128, 4 * HD], BF, tag="ptp",
                                                 name=f"ptp_{_rep}_{ci}")
                            for st in range(4):
                                kj = ci * 4 + st
                                nc.tensor.matmul(
                                    ptp[:, st * HD:(st + 1) * HD],
                                    vt_sb[:, kj * 128:(kj + 1) * 128],
                                    ident_sb[:, :],
                                    start=(st == 0), stop=(st == 3),
                                    is_transpose=True)
                            nc.scalar.copy(v_sb[:, ci * 4:(ci + 1) * 4, :],
                                           ptp[:, :])

                    # groups: (weight tile, cols, copy-out dest, name); k and
                    # q0 lead so head 0's attention inputs finish first, v
                    # third so vt is written before the q1-3 rope chain
                    groups = [
                        (wk_sb, bass.ds(0, HD), kt_sb[:, sl], "k"),
                        (wq_sb, bass.ds(0, HD), qt_sb[:, 0, sl], "q0"),
                        (wv_sb, bass.ds(0, HD), None, "v"),
                    ] + [
                        (wq_sb, bass.ds(h * HD, HD), qt_sb[:, h, sl], f"q{h}")
                        for h in range(1, NHL)
                    ]

                    def emit_block(gi, b, pp):
                        """6 DoubleRow matmuls covering ktiles 4b..4b+4 of
                        group gi: 2 hi@hi pair insts + 4 cross insts."""
                        w_sb, cols, _, _ = groups[gi]
                        first = (b == 0)
                        last = (b == HT // 4 - 1)
                        for j in range(2):          # ktile pairs
                            pt = 2 * b + j
                            t0 = 2 * pt
                            nc.tensor.matmul(
                                pp[:, :],
                                w_sb[:, t0:t0 + 2, 0, cols],
                                hs4[t0 // 4][:, (t0 % 4):(t0 % 4) + 2, 1, :],
                                start=(first and j == 0), stop=False,
                                perf_mode=DR)
                        for j in range(4):          # cross terms per ktile
                            t = 4 * b + j
                            nc.tensor.matmul(
                                pp[:, :],
                                w_sb[:, t, :, cols],
                                hs4[t // 4][:, t % 4, :, :],
                                start=False, stop=(last and j == 3),
                                perf_mode=DR)

                    if load_weights:
                        # chunk 0 is paced by the input DMA stream: interleave
                        # ALL 6 groups across arriving hst tiles so PE keeps
                        # up with the DMA rate. The attention PSUM pools are
                        # idle during chunk 0, so borrow their banks (every
                        # tile here is the same [128, CH] fp32 = 1 bank).
                        lenders = [(proj_pool, "pp"), (proj_pool, "pp"),
                                   (proj_pool, "pp"), (ps_pool, "ps"),
                                   (po_pool, "po"), (ps_pool, "ps")]
                        # k,v,q0 on proj; q1->ps, q2->po, q3->pd: the borrowed
                        # banks release in time for p2(0)'s first tiles
                        pps = [pool.tile([128, CH], F32, tag=tg,
                                         name=f"pp_{_rep}_{ci}_{g[3]}")
                               for (pool, tg), g in zip(lenders, groups)]
                        # interleaved while the DMA stream is the limiter
                        # (v deferred: its weights arrive last)...
                        B0 = 6
                        for b in range(B0):
                            for gi in (0, 1, 3, 4, 5):
                                emit_block(gi, b, pps[gi])
                        # ...then staggered tails so each group's copy-out
                        # chain overlaps the next group's matmuls; v's full
                        # block run goes third, against the stream's tail
                        for gi in (0, 1, 2, 3, 4, 5):
                            b0 = 0 if gi == 2 else B0
                            for b in range(b0, HT // 4):
                                emit_block(gi, b, pps[gi])
                            copy_out(pps[gi], groups[gi][2])
                    else:
                        for gi in range(6):
                            pp = proj_pool.tile(
                                [128, CH], F32, tag="pp",
                                name=f"pp_{_rep}_{ci}_{groups[gi][3]}")
                            for b in range(HT // 4):
                                emit_block(gi, b, pp)
                            copy_out(pp, groups[gi][2])

                def emit_p2(ci, filler=None, prefill=None):
                    """Attention for query chunk ci, all 4 local heads.
                    Emission is software-pipelined: scores(kj+1) is emitted
                    before attnV(kj) so PE computes the next score tile while
                    ACT does exp of the previous one. `filler` optionally
                    names one earlier-chunk output-projection s-tile to emit
                    after each head: attention alone is exp(ACT)-paced, so
                    the interleaved projection matmuls keep PE fed."""
                    q0 = ci * CH
                    n_kt = 4 * (ci + 1)
                    for h in range(NHL):
                        po = po_pool.tile([128, CH], F32,
                                          name=f"po_{_rep}_{ci}_{h}", tag="po")
                        # two denominator accumulators: even score tiles sum
                        # on DVE, odd on the (otherwise idle) Pool engine
                        dacc = dacc_pool.tile([128, CH], BF, tag="dacc",
                                              name=f"dacc_{_rep}_{ci}_{h}")
                        if not (ci == NCHUNK - 1 and h == NHL - 1):
                            dacp = dacc_pool.tile([128, CH], BF, tag="dacp",
                                                  name=f"dacp_{_rep}_{ci}_{h}")
                        stage = []  # (kj, ps, pr, off)
                        last_pr = []

                        def emit_scores(kj):
                            off = kj * 128 - q0
                            ps = ps_pool.tile([128, CH], F32,
                                              name=f"ps_{_rep}_{ci}_{h}_{kj}",
                                              tag="ps")
                            kt_t = kt_sb[:, kj * 128:(kj + 1) * 128]
                            if off > 0:
                                # columns < off are fully masked: skip them
                                nc.tensor.matmul(ps[:, off:],
                                                 kt_t, qt_sb[:, h, q0 + off:q0 + CH],
                                                 start=True, stop=True)
                            else:
                                nc.tensor.matmul(ps[:, :], kt_t,
                                                 qt_sb[:, h, q0:q0 + CH],
                                                 start=True, stop=True)
                            pr = probs_pool.tile([128, CH], BF,
                                                 name=f"pr_{_rep}_{ci}_{h}_{kj}",
                                                 tag="pr")
                            if off > 0:
                                nc.scalar.activation(
                                    pr[:, off:], ps[:, off:],
                                    mybir.ActivationFunctionType.Exp)
                            else:
                                nc.scalar.activation(
                                    pr[:, :], ps[:, :],
                                    mybir.ActivationFunctionType.Exp)
                            if off >= 0:
                                # triangular 128-col band at q_local in
                                # [off, off+128): maskb[:, 512:640] is the
                                # aligned triangle for every diagonal tile
                                nc.vector.tensor_mul(
                                    pr[:, off:off + 128], pr[:, off:off + 128],
                                    mask_sb[:, 512:640])
                            # denominator: accumulate probs tiles (DVE for
                            # even kj, Pool for odd) so the partition-sum
                            # needs only ONE matmul per (ci,h)
                            lo = max(off, 0)
                            dve_only = (ci == NCHUNK - 1 and h == NHL - 1)
                            if kj == 0:
                                # seed with exp(sink)/128: the all-ones
                                # partition-sum scales it back to exp(sink),
                                # so no separate denominator+sink add is
                                # needed
                                nc.vector.tensor_scalar_add(
                                    dacc[:, :], pr[:, :], esink_sb[:, h:h + 1])
                            elif dve_only:
                                if kj < n_kt - 1:
                                    nc.vector.tensor_add(dacc[:, lo:],
                                                         dacc[:, lo:],
                                                         pr[:, lo:])
                                else:
                                    last_pr[:] = [pr, lo]
                            elif kj == 2:
                                # pr[:, :off] is never written (fully masked
                                # region): seed only the valid columns
                                if lo > 0:
                                    nc.gpsimd.memset(dacp[:, 0:lo], 0.0)
                                    nc.gpsimd.tensor_copy(dacp[:, lo:],
                                                          pr[:, lo:])
                                else:
                                    nc.gpsimd.tensor_copy(dacp[:, :],
                                                          pr[:, :])
                            elif kj % 3 == 2:
                                # 1-in-3 on Pool: its ops cost ~2.1x DVE's
                                nc.gpsimd.tensor_add(dacp[:, lo:],
                                                     dacp[:, lo:], pr[:, lo:])
                            else:
                                nc.vector.tensor_add(dacc[:, lo:],
                                                     dacc[:, lo:], pr[:, lo:])
                            stage.append((kj, ps, pr, off))

                        def emit_av():
                            kj, ps, pr, off = stage.pop(0)
                            fl = dict(start=(kj == 0), stop=(kj == n_kt - 1))
                            if off > 0:
                                nc.tensor.matmul(po[:, off:], v_sb[:, kj, :],
                                                 pr[:, off:], **fl)
                            else:
                                nc.tensor.matmul(po[:, :], v_sb[:, kj, :],
                                                 pr[:, :], **fl)

                        emit_scores(0)
                        for kj in range(1, n_kt):
                            emit_scores(kj)
                            emit_av()
                        emit_av()

                        # partition-sum both accumulators straight into one
                        # PSUM group: a second 512-cycle matmul is cheaper
                        # than a cross-engine merge on the pd critical path
                        pd = ps_pool.tile([128, CH], F32, tag="ps",
                                          name=f"pd_{_rep}_{ci}_{h}")
                        last_head = (ci == NCHUNK - 1 and h == NHL - 1)
                        nc.tensor.matmul(pd[:, :], ones_sb[:, :], dacc[:, :],
                                         start=True, stop=False)
                        if last_head:
                            # the newest tile skips the accumulator: one hop
                            # less on the p3(3) critical chain
                            pr15, lo15 = last_pr[0], last_pr[1]
                            nc.tensor.matmul(pd[:, lo15:], ones_sb[:, :],
                                             pr15[:, lo15:],
                                             start=False, stop=True)
                        else:
                            nc.tensor.matmul(pd[:, :], ones_sb[:, :],
                                             dacp[:, :],
                                             start=False, stop=True)
                        rec = den_pool.tile([128, CH], F32, tag="rec",
                                            name=f"rec_{_rep}_{ci}_{h}")
                        nc.vector.reciprocal(rec[:, :], pd[:, :])
                        at_t = att_pool.tile([128, CH], BF, tag="att",
                                             name=f"att_{_rep}_{ci}_{h}")
                        nc.vector.tensor_mul(at_t[:, :], po[:, :], rec[:, :])
                        # hi/lo fp8 split for the DoubleRow output projection
                        # on Pool: keeps the busy DVE off this non-critical
                        # tail (the split is only needed by the next chunk's
                        # fillers)
                        at8 = at8a if h < 2 else at8b
                        hh2 = h % 2
                        if h == NHL - 1:
                            # last head: DVE (faster) -- its split gates the
                            # next phase's first output-projection tiles
                            nc.vector.tensor_copy(
                                at8[:, hh2, 0, q0:q0 + CH], at_t[:, :])
                            nc.vector.tensor_tensor(
                                at8[:, hh2, 1, q0:q0 + CH], at_t[:, :],
                                at8[:, hh2, 0, q0:q0 + CH],
                                mybir.AluOpType.subtract)
                        else:
                            nc.gpsimd.tensor_copy(
                                at8[:, hh2, 0, q0:q0 + CH], at_t[:, :])
                            nc.gpsimd.tensor_tensor(
                                at8[:, hh2, 1, q0:q0 + CH], at_t[:, :],
                                at8[:, hh2, 0, q0:q0 + CH],
                                mybir.AluOpType.subtract)
                        if filler is not None:
                            for fst, a, b in filler[h]:
                                emit_p3_st(fst, a, b)

                def emit_p3_st(st, oc0=0, oc1=H // CH, act_frac=4):
                    """Output projection for s-tile st (oc strips [oc0,oc1)):
                    compensated fp8 DoubleRow over the 4 local heads (2 hi@hi
                    pair insts + 4 cross insts per psum tile). PSUM comes
                    from the proj pool (idle whenever this runs). act_frac:
                    1-in-N copies go to ACT (use 2 when no exp pressure,
                    4 when interleaved with attention)."""
                    ssl = bass.ds(st * 128, 128)
                    # the very last tile ends with two per-512 strips so the
                    # final drain is one small DMA after a short copy
                    for oc in range(oc0, oc1):
                        osl = bass.ds(oc * CH, CH)
                        strip = 1 if (st == NKT - 1 and oc >= 6) else 2
                        o0 = oc - (oc % 2 if strip == 2 else 0)
                        if oc == o0:
                            ob = out_pool.tile(
                                [128, strip * CH], BF, tag=f"ob{strip}",
                                name=f"ob_{_rep}_{st}_{oc}")
                        pw = proj_pool.tile([128, CH], F32, tag="pp",
                                            name=f"pw_{_rep}_{st}_{oc}")
                        for j in range(2):
                            nc.tensor.matmul(
                                pw[:, :],
                                (at8a if j == 0 else at8b)[:, :, 0, ssl],
                                wo_sb[:, 2 * j:2 * j + 2, 1, osl],
                                start=(j == 0), stop=False, perf_mode=DR)
                        for hh in range(NHL):
                            nc.tensor.matmul(
                                pw[:, :],
                                (at8a if hh < 2 else at8b)[:, hh % 2, :, ssl],
                                wo_sb[:, hh, :, osl],
                                start=False, stop=(hh == NHL - 1),
                                perf_mode=DR)
                        # alternate copy engine to split PSUM->SBUF load;
                        # descale the fp8 weight prescale
                        half = bass.ds((oc - o0) * CH, CH)
                        if act_frac == 1 or (st * (H // CH) + oc) \
                                % act_frac == act_frac - 1:
                            nc.scalar.mul(ob[:, half], pw[:, :], DESC)
                        else:
                            nc.vector.tensor_scalar_mul(
                                ob[:, half], pw[:, :], DESC)
                        if oc == o0 + strip - 1:
                            # strips emitted before the last hst chunk go on
                            # the ACT queue: on SP their wait for the ob copy
                            # would head-block chunk-3's hst stream
                            dma = (nc.scalar.dma_start
                                   if (st < 4 or (st == NKT - 1 and oc % 2))
                                   else nc.sync.dma_start)
                            dma(out=outp[st * 128:(st + 1) * 128,
                                         o0 * CH:(o0 + strip) * CH],
                                in_=ob[:, :])

                def emit_p3_staggered(st):
                    """First tile after a chunk's attention: the at8b pair
                    still waits on the last head's hi/lo split, so emit the
                    at8a-dependent halves of three psum groups first."""
                    ssl = bass.ds(st * 128, 128)
                    pws = []
                    lend = [(proj_pool, "pp"), (proj_pool, "pp"),
                            (proj_pool, "pp"), (ps_pool, "ps")]
                    for oc in range(4):
                        osl = bass.ds(oc * CH, CH)
                        pw = lend[oc][0].tile([128, CH], F32, tag=lend[oc][1],
                                              name=f"pw_{_rep}_{st}_{oc}")
                        nc.tensor.matmul(pw[:, :], at8a[:, :, 0, ssl],
                                         wo_sb[:, 0:2, 1, osl],
                                         start=True, stop=False, perf_mode=DR)
                        for hh in range(2):
                            nc.tensor.matmul(pw[:, :],
                                             at8a[:, hh, :, ssl],
                                             wo_sb[:, hh, :, osl],
                                             start=False, stop=False,
                                             perf_mode=DR)
                        pws.append(pw)
                    for oc in range(4):
                        osl = bass.ds(oc * CH, CH)
                        pw = pws[oc]
                        if oc % 2 == 0:
                            ob = out_pool.tile([128, 2 * CH], BF, tag="ob2",
                                               name=f"ob_{_rep}_{st}_{oc}")
                            obs = ob
                        else:
                            ob = obs
                        nc.tensor.matmul(pw[:, :], at8b[:, :, 0, ssl],
                                         wo_sb[:, 2:4, 1, osl],
                                         start=False, stop=False, perf_mode=DR)
                        for hh in range(2):
                            nc.tensor.matmul(pw[:, :],
                                             at8b[:, hh, :, ssl],
                                             wo_sb[:, 2 + hh, :, osl],
                                             start=False, stop=(hh == 1),
                                             perf_mode=DR)
                        half = bass.ds((oc % 2) * CH, CH)
                        if (st * (H // CH) + oc) % 4 == 3:
                            nc.scalar.mul(ob[:, half], pw[:, :], DESC)
                        else:
                            nc.vector.tensor_scalar_mul(
                                ob[:, half], pw[:, :], DESC)
                        if oc % 2 == 1:
                            nc.sync.dma_start(
                                out=outp[st * 128:(st + 1) * 128,
                                         (oc - 1) * CH:(oc + 1) * CH],
                                in_=ob[:, :])
                    emit_p3_st(st, 4, H // CH)

                # Phase order = PE program order. Chunk-0 attention directly
                # follows chunk-0 projections (fills the chunk-1 hst DMA
                # window); later attention chunks interleave one output-
                # projection s-tile per head so the exp-paced stretches keep
                # PE fed; p3 uses the proj PSUM bufs, which are free during
                # every p2/p3 stretch.
                def lagged(s0):
                    """Half-tile-lagged filler: head h gets the back half of
                    tile s0+h-1 and the front half of s0+h, so the last
                    head's DVE/ACT tail is covered by the leftover back half
                    emitted right after the chunk."""
                    oc4 = H // CH // 2
                    fill = [[(s0, 0, oc4)]]
                    for h in range(1, NHL):
                        fill.append([(s0 + h - 1, oc4, 2 * oc4),
                                     (s0 + h, 0, oc4)])
                    return fill

                emit_p1(0, load_weights=(_rep == 0))
                emit_p2(0)
                emit_p1(1)
                if _rep == 0:
                    nc.sync.dma_start(out=wo_sb[:, 0:2], in_=wo_r[:, 0:2])
                    nc.sync.dma_start(out=wo_sb[:, 2:4], in_=wo_r[:, 2:4])
                emit_p1(2)
                emit_p2(1, filler=lagged(0), prefill=0)
                emit_p3_st(3, H // CH // 2, H // CH)
                emit_p1(3)
                emit_p2(2, filler=lagged(4), prefill=4)
                emit_p3_st(7, H // CH // 2, H // CH)
                emit_p2(3, filler=lagged(8), prefill=8)
                emit_p3_st(11, H // CH // 2, H // CH)
                emit_p3_staggered(12)
                for st in range(13, 16):
                    emit_p3_st(st)

    _split_excess_waits(nc)
    return nc


_NC_CACHE = None


def _get_nc():
    global _NC_CACHE
    if _NC_CACHE is None:
        _NC_CACHE = build_bass()
    return _NC_CACHE


def _hilo_rows(x, order):
    """x [K, M] fp32 (pre-scaled) -> [(ktile p two), M] f8 with the residual
    split; order 'hilo' for stationary/lhsT tensors, 'lohi' for moving/rhs."""
    hi = x.astype(f8)
    lo = (x - hi.astype(np.float32)).astype(f8)
    a, b = (hi, lo) if order == "hilo" else (lo, hi)
    kt = x.shape[0] // 128
    out = np.empty((kt, 128, 2, x.shape[1]), dtype=f8)
    out[:, :, 0] = a.reshape(kt, 128, -1)
    out[:, :, 1] = b.reshape(kt, 128, -1)
    return out.reshape(kt * 128 * 2, x.shape[1])


def make_in_maps(hidden_states, cos, sin, Wq, Wk, Wv, Wo, sinks):
    scaling = HD ** -0.5
    hs = np.asarray(hidden_states, dtype=np.float32).reshape(S, H)
    hsT = np.ascontiguousarray(hs.T)
    # hs8: rows (ktile, partition), cols (chunk, two=(lo,hi), s-in-chunk)
    hi = hsT.astype(f8)
    lo = (hsT - hi.astype(np.float32)).astype(f8)
    A = np.empty((HT, 128, NCHUNK, 2, CH), dtype=f8)
    A[:, :, :, 0] = lo.reshape(HT, 128, NCHUNK, CH)
    A[:, :, :, 1] = hi.reshape(HT, 128, NCHUNK, CH)
    hs8 = A.reshape(HT * 128, 2 * S)
    cosT = np.ascontiguousarray(np.asarray(cos, np.float32).reshape(S, ROPE).T)
    sinT = np.ascontiguousarray(np.asarray(sin, np.float32).reshape(S, ROPE).T)
    sinTs = sinT.copy()
    sinTs[:ROPE // 2] *= -1.0
    cosT = (cosT * DESC).astype(bf16)
    sinTs = (sinTs * DESC).astype(bf16)
    Wq = np.asarray(Wq, np.float32)
    Wk = np.asarray(Wk, np.float32)
    Wv = np.asarray(Wv, np.float32)
    Wo = np.asarray(Wo, np.float32)
    sinks = np.asarray(sinks, np.float32)
    maskb = ((np.arange(1024)[None, :] - 512) >= np.arange(128)[:, None])
    maskb = maskb.astype(np.float32).astype(bf16)
    identity = np.eye(128, dtype=np.float32).astype(bf16)

    in_maps = []
    for c in range(N_CORES):
        qcols = slice(NHL * HD * c, NHL * HD * (c + 1))
        kcols = slice(HD * c, HD * (c + 1))
        esink_c = (np.exp(sinks[NHL * c:NHL * (c + 1)]) / 128.0).astype(np.float32)
        in_maps.append({
            "hs8": hs8,
            "wq8": _hilo_rows(Wq[:, qcols] * (scaling * SW), "hilo"),
            "wk8": _hilo_rows(Wk[:, kcols] * SW, "hilo"),
            "wv8": _hilo_rows(Wv[:, kcols] * SW, "hilo"),
            "wo8": _hilo_rows(Wo[qcols, :] * SW, "lohi"),
            "cosT": cosT,
            "sinTs": sinTs,
            "esink": np.repeat(esink_c[None, :], 128, axis=0).copy(),
            "maskb": maskb,
            "ident": identity,
        })
    return in_maps


def kernel(hidden_states, cos, sin, attention_mask, Wq, Wk, Wv, Wo, sinks):
    # attention_mask is the standard causal mask; causality is built into the
    # kernel (binary masks on the diagonal score tiles), so it is unused.
    in_maps = make_in_maps(hidden_states, cos, sin, Wq, Wk, Wv, Wo, sinks)
    nc = _get_nc()
    res = run_bass_kernel_spmd(nc, in_maps, core_ids=list(range(N_CORES)))
    acc = np.zeros((S, H), dtype=np.float32)
    for r in res.results:
        acc += r["outp"].astype(np.float32)
    return acc.reshape(1, S, H)


# revision 93
# speedup vs baseline: 1.0002x; 1.0002x over previous
"""MiMoV2 attention (GQA + partial RoPE + attention sinks + causal) on 8 TRN2
NeuronCores.

Sharding: tensor-parallel over heads. Core c owns KV head c and query heads
[4c, 4c+4). Wq/Wk/Wv split along output dim, Wo along input dim; each core
computes a partial output [S, H] which the host sums (the Wo contraction over
heads distributes over cores).

Per-core dataflow (everything head-transposed so no on-chip transposes needed):
  hsT [H, S] streamed by 128-row h-tiles; per s-chunk of 512:
    QT[d, s] (4 heads), KT[d, s] accumulate in PSUM over the h-tiles
    V[s, d] natural layout via hsT-as-stationary matmuls

  The QKV and output projections run as error-compensated fp8 matmuls in
  DoubleRow perf mode: x ~= x_hi + x_lo (both f8e4m3, residual split), and
  x@w = xh@wh + (xh@wl + xl@wh), dropping the lo*lo term (~0.07% error,
  below bf16 noise). The hi@hi pass packs two 128-deep k-tiles per DoubleRow
  instruction; the cross pass packs both cross products of one k-tile per
  instruction (w stored (hi,lo), x stored (lo,hi)). 3 passes over 2x-rate
  fp8 = 0.75x the PE cycles of bf16 at ~equal accuracy. Weights are
  pre-scaled by 512 on the host (fp8 subnormal floor), descaled in the
  PSUM->SBUF copy-outs (rope cos/sin pre-scaled, ACT copy-with-scale
  elsewhere).

  partial RoPE applied in [d, s] layout; rotate_half reads the PSUM rows
  cross-partition with the sign folded into sinT. V transposes to [s, d]
  via PE transpose matmuls (HWDGE generation is a serial resource the hst
  stream needs). scoresT[s_k, s_q] = KT_tile^T @ QT-chunk (bf16: with a
  128-deep contraction, 3-pass fp8 DoubleRow would cost 1.5x, so bf16 is
  optimal here); exp on ACT (no max subtraction -- |scores| <= ~12);
  causal via binary mask multiply on diagonal tiles only. attn_outT[d,
  s_q] accumulates V_tile^T @ probsT (bf16). The softmax denominator
  accumulates the probs tiles on DVE and Pool (2:1, matching their
  throughputs) seeded with exp(sink)/128, and finishes with two all-ones
  matmuls per (chunk, head) -- vs one per score tile -- freeing ~8% of PE
  cycles; division by DVE reciprocal + multiply, then the attnT chunk is
  split hi/lo fp8 (Pool; DVE for the head gating the final phase).
  out_partial[s, o] = attnT-as-stationary @ Wo-chunk (fp8 DoubleRow over
  the 4 local heads), interleaved tile-by-tile into the exp-paced
  attention chunks; written out as bf16 in 1024-col strips; host sums the
  8 partials in fp32.

  Emission order is the PE schedule: p1(0), p2(0), p1(1), p1(2),
  p2(1)+p3(0), p1(3), p2(2)+p3(1), p2(3)+p3(2), p3(3), with p3 s-tiles
  half-lagged behind the heads so the last head's softmax tail overlaps
  projection matmuls.
"""

import numpy as np
import ml_dtypes
from contextlib import ExitStack

import concourse.bass as bass
import concourse.mybir as mybir
import concourse.tile as tile
from concourse.bass_utils import run_bass_kernel_spmd

bf16 = ml_dtypes.bfloat16
f8 = ml_dtypes.float8_e4m3
BF = mybir.dt.bfloat16
F32 = mybir.dt.float32
F8 = mybir.dt.float8e4
DR = mybir.MatmulPerfMode.DoubleRow

N_CORES = 8
S = 2048
H = 4096
HD = 128
ROPE = 64
NHL = 4                    # local query heads per core
CH = 512                   # s-chunk width
NCHUNK = S // CH           # 4
HT = H // 128              # 32 h-tiles
NKT = S // 128             # 16 k-tiles
SW = 512.0                 # host-side fp8 weight pre-scale (power of 2)
DESC = 1.0 / SW

# this walrus build allows at most one sync wait per instruction
_MAX_WAITS = 1


def _split_excess_waits(nc):
    cnt = 0
    for f in nc.m.functions:
        for bb in f.blocks:
            out, changed = [], False
            for inst in bb.instructions:
                si = inst.sync_info
                if si is not None and len(si.on_wait) > _MAX_WAITS:
                    waits = list(si.on_wait)
                    excess, keep = waits[:-_MAX_WAITS], waits[-_MAX_WAITS:]
                    for i in range(0, len(excess), _MAX_WAITS):
                        cnt += 1
                        out.append(mybir.InstNoOp(
                            name=f"waitnop-{cnt}", engine=inst.engine,
                            sync_info=mybir.SyncInfo(
                                on_wait=excess[i:i + _MAX_WAITS], on_update=[])))
                    si.on_wait = keep
                    changed = True
                out.append(inst)
            if changed:
                bb.instructions = out
    return cnt


def _rope_copy(nc, pool, psum_t, dest, cos_sb, sin_sb, sl):
    """psum_t [128,512] fp32 -> dest [128,512] bf16 slice, applying partial
    RoPE to rows 0:64 (rotate_half = +-32-partition swap done by reading the
    PSUM rows cross-partition, sign pre-folded into sin_sb). cos/sin are
    pre-scaled by DESC on the host, and the pass-through copy descales on
    ACT, so the fp8 weight scale cancels here.
    """
    # pass-through rows 64:128 on ACT (keeps DVE free), descaled
    nc.scalar.mul(dest[64:128, :], psum_t[64:128, :], DESC)
    # t2 = rotate_half(q_r) * sin, reading the swapped halves straight from
    # PSUM (cross-partition operand offsets)
    t2 = pool.tile([64, CH], BF, tag="rope_t2")
    nc.vector.tensor_mul(t2[0:32, :], psum_t[32:64, :], sin_sb[0:32, sl])
    nc.vector.tensor_mul(t2[32:64, :], psum_t[0:32, :], sin_sb[32:64, sl])
    # t1 = q_r * cos   (one fused op: (psum mult 1.0) mult cos)
    t1 = pool.tile([64, CH], BF, tag="rope_t1")
    nc.vector.scalar_tensor_tensor(
        t1[:, :], psum_t[0:64, :], 1.0, cos_sb[:, sl],
        op0=mybir.AluOpType.mult, op1=mybir.AluOpType.mult)
    nc.vector.tensor_add(dest[0:64, :], t1[:, :], t2[:, :])


def build_bass(repeat=1):
    """repeat>1 duplicates the whole compute body (for timing: the wall-clock
    delta between repeat=2 and repeat=1 NEFFs is one kernel iteration,
    independent of the large fixed PJRT/axon dispatch overhead)."""
    nc = bass.Bass("TRN2", target_bir_lowering=False, debug=False)

    # fp8 hi/lo-split tensors: `two` holds (lo, hi) for moving/rhs-side
    # tensors and (hi, lo) for stationary/lhsT-side tensors so one DoubleRow
    # instruction covers both cross terms. hs8 rows are (ktile, partition)
    # and columns (chunk, two, s) so one 3-dim DMA moves a whole
    # [4-ktile, both-plane, chunk] tile.
    hs8 = nc.dram_tensor("hs8", [HT * 128, 2 * S], F8, kind="ExternalInput")
    wq8 = nc.dram_tensor("wq8", [HT * 128 * 2, NHL * HD], F8, kind="ExternalInput")
    wk8 = nc.dram_tensor("wk8", [HT * 128 * 2, HD], F8, kind="ExternalInput")
    wv8 = nc.dram_tensor("wv8", [HT * 128 * 2, HD], F8, kind="ExternalInput")
    wo8 = nc.dram_tensor("wo8", [NHL * 128 * 2, H], F8, kind="ExternalInput")
    cosT = nc.dram_tensor("cosT", [ROPE, S], BF, kind="ExternalInput")
    sinTs = nc.dram_tensor("sinTs", [ROPE, S], BF, kind="ExternalInput")
    esink = nc.dram_tensor("esink", [128, NHL], F32, kind="ExternalInput")
    ident = nc.dram_tensor("ident", [128, 128], BF, kind="ExternalInput")
    maskb = nc.dram_tensor("maskb", [128, 1024], BF, kind="ExternalInput")
    outp = nc.dram_tensor("outp", [S, H], BF, kind="ExternalOutput")

    with tile.TileContext(nc) as tc, ExitStack() as ctx:
        const = ctx.enter_context(tc.tile_pool(name="const", bufs=1))
        hs_pool = ctx.enter_context(tc.tile_pool(name="hs", bufs=9))
        rope_pool = ctx.enter_context(tc.tile_pool(name="rope", bufs=2))
        probs_pool = ctx.enter_context(tc.tile_pool(name="probs", bufs=9))
        den_pool = ctx.enter_context(tc.tile_pool(name="den", bufs=2))
        dacc_pool = ctx.enter_context(tc.tile_pool(name="dacc", bufs=2))
        att_pool = ctx.enter_context(tc.tile_pool(name="att", bufs=3))
        out_pool = ctx.enter_context(tc.tile_pool(name="out", bufs=4))

        # ---- constants / weights resident in SBUF ----
        # weights are loaded in h-tile groups so the first projection matmuls
        # only wait on the first slice, not the whole tensor
        wq_sb = const.tile([128, HT, 2, NHL * HD], F8)
        wk_sb = const.tile([128, HT, 2, HD], F8)
        wv_sb = const.tile([128, HT, 2, HD], F8)
        wq_r = wq8.rearrange("(t p two) c -> p t two c", p=128, two=2)
        wk_r = wk8.rearrange("(t p two) c -> p t two c", p=128, two=2)
        wv_r = wv8.rearrange("(t p two) c -> p t two c", p=128, two=2)
        hsT_r = hs8.rearrange("(t p) (c x) -> p t c x", p=128, c=NCHUNK)
        wo_sb = const.tile([128, NHL, 2, H], F8)
        wo_r = wo8.rearrange("(t p two) c -> p t two c", p=128, two=2)
        cos_sb = const.tile([ROPE, S], BF)
        sin_sb = const.tile([ROPE, S], BF)
        mask_sb = const.tile([128, 1024], BF)
        esink_sb = const.tile([128, NHL], F32)
        nc.gpsimd.dma_start(out=esink_sb, in_=esink[:, :])
        ident_sb = const.tile([128, 128], BF)
        nc.gpsimd.dma_start(out=ident_sb, in_=ident[:, :])
        ones_sb = const.tile([128, 128], BF)
        nc.vector.memset(ones_sb[:, :], 1.0)

        # persistent activations
        qt_sb = const.tile([128, NHL, S], BF)     # QT per head [d, s]
        kt_sb = const.tile([128, S], BF)          # KT [d, s]
        vt_sb = const.tile([128, S], BF)          # VT [d, s] (pre-transpose)
        v_sb = const.tile([128, NKT, HD], BF)     # V [s(128), kt, d]
        # attnT (hi,lo) [d, s], split by head PAIR: coarse per-tile write
        # tracking otherwise makes every p3 matmul wait on the latest head's
        # split (the hi@hi insts need each pair adjacent, so no finer split)
        at8a = const.tile([128, 2, 2, S], F8)
        at8b = const.tile([128, 2, 2, S], F8)

        for _rep in range(repeat):
            # phases 1+2 share one PSUM scope (8 banks: proj 3 + ps 2 + po 2
            # + pd 1) so projection chunks and attention chunks interleave on
            # PE with no pool-boundary serialization.
            with ExitStack() as p12:
                proj_pool = p12.enter_context(
                    tc.tile_pool(name="proj", bufs=3, space="PSUM"))
                ps_pool = p12.enter_context(
                    tc.tile_pool(name="ps", bufs=2, space="PSUM"))
                po_pool = p12.enter_context(
                    tc.tile_pool(name="po", bufs=2, space="PSUM"))
                ptp_pool = p12.enter_context(
                    tc.tile_pool(name="ptp", bufs=1, space="PSUM"))

                def emit_p1(ci, load_weights=False):
                    """QKV projections + RoPE for s-chunk ci. Each group's 48
                    DoubleRow matmuls (16 hi@hi ktile-pairs + 32 cross) form
                    one contiguous PSUM accumulation group in a single bank."""
                    sl = bass.ds(ci * CH, CH)
                    hs4 = []
                    for g4 in range(HT // 4):
                        h4 = hs_pool.tile([128, 4, 2, CH], F8, tag="hst",
                                          name=f"hst_{_rep}_{ci}_{g4}")
                        g = g4 * 4
                        # DMA APs allow max 3 dims: the s-chunk slice blocks
                        # (ktile,two) merging, so move each fp8 plane
                        # separately. The hst stream owns the SP queue; the
                        # weights go on the Pool queue so neither blocks the
                        # other at the queue head.
                        if load_weights and g4 % 2 == 0:
                            # interleave weight-slice loads with the hst
                            # stream (k first: the first matmuls are group
                            # k's) so the first matmuls start early
                            nc.sync.dma_start(out=wk_sb[:, g:g + 8],
                                              in_=wk_r[:, g:g + 8])
                        nc.sync.dma_start(out=h4, in_=hsT_r[:, g:g + 4, ci, :])
                        if load_weights and g4 in (0, 1, 6):
                            if g4 == 6:
                                # the tail of the stream is latency-critical:
                                # split the last wq batch so block 6 doesn't
                                # wait on the full 1MB transfer
                                nc.sync.dma_start(out=wq_sb[:, 24:28],
                                                  in_=wq_r[:, 24:28])
                                nc.sync.dma_start(out=wq_sb[:, 28:32],
                                                  in_=wq_r[:, 28:32])
                            elif g4 == 0:
                                # head of the stream likewise: q0's first
                                # block waits on wq, and hst-g1 queues behind
                                nc.sync.dma_start(out=wq_sb[:, 0:4],
                                                  in_=wq_r[:, 0:4])
                            else:
                                nc.sync.dma_start(out=wq_sb[:, 4:8],
                                                  in_=wq_r[:, 4:8])
                        elif load_weights and g4 % 2 == 0:
                            nc.sync.dma_start(out=wq_sb[:, g:g + 8],
                                              in_=wq_r[:, g:g + 8])
                        if load_weights:
                            if g4 == 5:
                                # rope/mask constants: queued behind the
                                # critical weight stream, ready well
                                # before the first rope copy-out
                                nc.sync.dma_start(out=cos_sb,
                                                  in_=cosT[:, :])
                                nc.sync.dma_start(out=sin_sb,
                                                  in_=sinTs[:, :])
                                nc.sync.dma_start(out=mask_sb,
                                                  in_=maskb[:, :])
                        hs4.append(h4)
                    if load_weights:
                        # wv rides at the end of the stream: the v group's
                        # matmuls are deferred past the interleave, so its
                        # 1MB stays off the critical early window
                        nc.sync.dma_start(out=wv_sb[:, 0:16],
                                          in_=wv_r[:, 0:16])
                        nc.sync.dma_start(out=wv_sb[:, 16:32],
                                          in_=wv_r[:, 16:32])

                    def copy_out(pp, rope):
                        if rope is not None:
                            _rope_copy(nc, rope_pool, pp, rope, cos_sb, sin_sb, sl)
                        else:
                            nc.vector.tensor_scalar_mul(vt_sb[:, sl], pp[:, :],
                                                        DESC)
                            # transpose to [s, d] on PE (53ns/tile) instead of
                            # 4 DMAs: HWDGE generation is a serial resource
                            # the hst stream needs
                            ptp = ptp_pool.tile([128, 4 * HD], BF, tag="ptp",
                                                 name=f"ptp_{_rep}_{ci}")
                            for st in range(4):
                                kj = ci * 4 + st
                                nc.tensor.matmul(
                                    ptp[:, st * HD:(st + 1) * HD],
                                    vt_sb[:, kj * 128:(kj + 1) * 128],
                                    ident_sb[:, :],
                                    start=(st == 0), stop=(st == 3),
                                    is_transpose=True)
                            nc.scalar.copy(v_sb[:, ci * 4:(ci + 1) * 4, :],
                                           ptp[:, :])

                    # groups: (weight tile, cols, copy-out dest, name); k and
                    # q0 lead so head 0's attention inputs finish first, v
                    # third so vt is written before the q1-3 rope chain
                    groups = [
                        (wk_sb, bass.ds(0, HD), kt_sb[:, sl], "k"),
                        (wq_sb, bass.ds(0, HD), qt_sb[:, 0, sl], "q0"),
                        (wv_sb, bass.ds(0, HD), None, "v"),
                    ] + [
                        (wq_sb, bass.ds(h * HD, HD), qt_sb[:, h, sl], f"q{h}")
                        for h in range(1, NHL)
                    ]

                    def emit_block(gi, b, pp):
                        """6 DoubleRow matmuls covering ktiles 4b..4b+4 of
                        group gi: 2 hi@hi pair insts + 4 cross insts."""
                        w_sb, cols, _, _ = groups[gi]
                        first = (b == 0)
                        last = (b == HT // 4 - 1)
                        for j in range(2):          # ktile pairs
                            pt = 2 * b + j
                            t0 = 2 * pt
                            nc.tensor.matmul(
                                pp[:, :],
                                w_sb[:, t0:t0 + 2, 0, cols],
                                hs4[t0 // 4][:, (t0 % 4):(t0 % 4) + 2, 1, :],
                                start=(first and j == 0), stop=False,
                                perf_mode=DR)
                        for j in range(4):          # cross terms per ktile
                            t = 4 * b + j
                            nc.tensor.matmul(
                                pp[:, :],
                                w_sb[:, t, :, cols],
                                hs4[t // 4][:, t % 4, :, :],
                                start=False, stop=(last and j == 3),
                                perf_mode=DR)

                    if load_weights:
                        # chunk 0 is paced by the input DMA stream: interleave
                        # ALL 6 groups across arriving hst tiles so PE keeps
                        # up with the DMA rate. The attention PSUM pools are
                        # idle during chunk 0, so borrow their banks (every
                        # tile here is the same [128, CH] fp32 = 1 bank).
                        lenders = [(proj_pool, "pp"), (proj_pool, "pp"),
                                   (proj_pool, "pp"), (ps_pool, "ps"),
                                   (po_pool, "po"), (ps_pool, "ps")]
                        # k,v,q0 on proj; q1->ps, q2->po, q3->pd: the borrowed
                        # banks release in time for p2(0)'s first tiles
                        pps = [pool.tile([128, CH], F32, tag=tg,
                                         name=f"pp_{_rep}_{ci}_{g[3]}")
                               for (pool, tg), g in zip(lenders, groups)]
                        # interleaved while the DMA stream is the limiter
                        # (v deferred: its weights arrive last)...
                        B0 = 6
                        for b in range(B0):
                            for gi in (0, 1, 3, 4, 5):
                                emit_block(gi, b, pps[gi])
                        # ...then staggered tails so each group's copy-out
                        # chain overlaps the next group's matmuls; v's full
                        # block run goes third, against the stream's tail
                        for gi in (0, 1, 2, 3, 4, 5):
                            b0 = 0 if gi == 2 else B0
                            for b in range(b0, HT // 4):
                                emit_block(gi, b, pps[gi])
                            copy_out(pps[gi], groups[gi][2])
                    else:
                        for gi in range(6):
                            pp = proj_pool.tile(
                                [128, CH], F32, tag="pp",
                                name=f"pp_{_rep}_{ci}_{groups[gi][3]}")
                            for b in range(HT // 4):
                                emit_block(gi, b, pp)
                            copy_out(pp, groups[gi][2])

                def emit_p2(ci, filler=None, prefill=None):
                    """Attention for query chunk ci, all 4 local heads.
                    Emission is software-pipelined: scores(kj+1) is emitted
                    before attnV(kj) so PE computes the next score tile while
                    ACT does exp of the previous one. `filler` optionally
                    names one earlier-chunk output-projection s-tile to emit
                    after each head: attention alone is exp(ACT)-paced, so
                    the interleaved projection matmuls keep PE fed."""
                    q0 = ci * CH
                    n_kt = 4 * (ci + 1)
                    for h in range(NHL):
                        po = po_pool.tile([128, CH], F32,
                                          name=f"po_{_rep}_{ci}_{h}", tag="po")
                        # two denominator accumulators: even score tiles sum
                        # on DVE, odd on the (otherwise idle) Pool engine
                        dacc = dacc_pool.tile([128, CH], BF, tag="dacc",
                                              name=f"dacc_{_rep}_{ci}_{h}")
                        if not (ci == NCHUNK - 1 and h == NHL - 1):
                            dacp = dacc_pool.tile([128, CH], BF, tag="dacp",
                                                  name=f"dacp_{_rep}_{ci}_{h}")
                        stage = []  # (kj, ps, pr, off)
                        last_pr = []

                        def emit_scores(kj):
                            off = kj * 128 - q0
                            ps = ps_pool.tile([128, CH], F32,
                                              name=f"ps_{_rep}_{ci}_{h}_{kj}",
                                              tag="ps")
                            kt_t = kt_sb[:, kj * 128:(kj + 1) * 128]
                            if off > 0:
                                # columns < off are fully masked: skip them
                                nc.tensor.matmul(ps[:, off:],
                                                 kt_t, qt_sb[:, h, q0 + off:q0 + CH],
                                                 start=True, stop=True)
                            else:
                                nc.tensor.matmul(ps[:, :], kt_t,
                                                 qt_sb[:, h, q0:q0 + CH],
                                                 start=True, stop=True)
                            pr = probs_pool.tile([128, CH], BF,
                                                 name=f"pr_{_rep}_{ci}_{h}_{kj}",
                                                 tag="pr")
                            if off > 0:
                                nc.scalar.activation(
                                    pr[:, off:], ps[:, off:],
                                    mybir.ActivationFunctionType.Exp)
                            else:
                                nc.scalar.activation(
                                    pr[:, :], ps[:, :],
                                    mybir.ActivationFunctionType.Exp)
                            if off >= 0:
                                # triangular 128-col band at q_local in
                                # [off, off+128): maskb[:, 512:640] is the
                                # aligned triangle for every diagonal tile
                                nc.vector.tensor_mul(
                                    pr[:, off:off + 128], pr[:, off:off + 128],
                                    mask_sb[:, 512:640])
                            # denominator: accumulate probs tiles (DVE for
                            # even kj, Pool for odd) so the partition-sum
                            # needs only ONE matmul per (ci,h)
                            lo = max(off, 0)
                            dve_only = (ci == NCHUNK - 1 and h == NHL - 1)
                            if kj == 0:
                                # seed with exp(sink)/128: the all-ones
                                # partition-sum scales it back to exp(sink),
                                # so no separate denominator+sink add is
                                # needed
                                nc.vector.tensor_scalar_add(
                                    dacc[:, :], pr[:, :], esink_sb[:, h:h + 1])
                            elif dve_only:
                                if kj < n_kt - 1:
                                    nc.vector.tensor_add(dacc[:, lo:],
                                                         dacc[:, lo:],
                                                         pr[:, lo:])
                                else:
                                    last_pr[:] = [pr, lo]
                            elif kj == 2:
                                # pr[:, :off] is never written (fully masked
                                # region): seed only the valid columns
                                if lo > 0:
                                    nc.gpsimd.memset(dacp[:, 0:lo], 0.0)
                                    nc.gpsimd.tensor_copy(dacp[:, lo:],
                                                          pr[:, lo:])
                                else:
                                    nc.gpsimd.tensor_copy(dacp[:, :],
                                                          pr[:, :])
                            elif kj % 3 == 2:
                                # 1-in-3 on Pool: its ops cost ~2.1x DVE's
                                nc.gpsimd.tensor_add(dacp[:, lo:],
                                                     dacp[:, lo:], pr[:, lo:])
                            else:
                                nc.vector.tensor_add(dacc[:, lo:],
                                                     dacc[:, lo:], pr[:, lo:])
                            stage.append((kj, ps, pr, off))

                        def emit_av():
                            kj, ps, pr, off = stage.pop(0)
                            fl = dict(start=(kj == 0), stop=(kj == n_kt - 1))
                            if off > 0:
                                nc.tensor.matmul(po[:, off:], v_sb[:, kj, :],
                                                 pr[:, off:], **fl)
                            else:
                                nc.tensor.matmul(po[:, :], v_sb[:, kj, :],
                                                 pr[:, :], **fl)

                        emit_scores(0)
                        for kj in range(1, n_kt):
                            emit_scores(kj)
                            emit_av()
                        emit_av()

                        # partition-sum both accumulators straight into one
                        # PSUM group: a second 512-cycle matmul is cheaper
                        # than a cross-engine merge on the pd critical path
                        pd = ps_pool.tile([128, CH], F32, tag="ps",
                                          name=f"pd_{_rep}_{ci}_{h}")
                        last_head = (ci == NCHUNK - 1 and h == NHL - 1)
                        nc.tensor.matmul(pd[:, :], ones_sb[:, :], dacc[:, :],
                                         start=True, stop=False)
                        if last_head:
                            # the newest tile skips the accumulator: one hop
                            # less on the p3(3) critical chain
                            pr15, lo15 = last_pr[0], last_pr[1]
                            nc.tensor.matmul(pd[:, lo15:], ones_sb[:, :],
                                             pr15[:, lo15:],
                                             start=False, stop=True)
                        else:
                            nc.tensor.matmul(pd[:, :], ones_sb[:, :],
                                             dacp[:, :],
                                             start=False, stop=True)
                        rec = den_pool.tile([128, CH], F32, tag="rec",
                                            name=f"rec_{_rep}_{ci}_{h}")
                        nc.vector.reciprocal(rec[:, :], pd[:, :])
                        at_t = att_pool.tile([128, CH], BF, tag="att",
                                             name=f"att_{_rep}_{ci}_{h}")
                        nc.vector.tensor_mul(at_t[:, :], po[:, :], rec[:, :])
                        # hi/lo fp8 split for the DoubleRow output projection
                        # on Pool: keeps the busy DVE off this non-critical
                        # tail (the split is only needed by the next chunk's
                        # fillers)
                        at8 = at8a if h < 2 else at8b
                        hh2 = h % 2
                        if h == NHL - 1:
                            # last head: DVE (faster) -- its split gates the
                            # next phase's first output-projection tiles
                            nc.vector.tensor_copy(
                                at8[:, hh2, 0, q0:q0 + CH], at_t[:, :])
                            nc.vector.tensor_tensor(
                                at8[:, hh2, 1, q0:q0 + CH], at_t[:, :],
                                at8[:, hh2, 0, q0:q0 + CH],
                                mybir.AluOpType.subtract)
                        else:
                            nc.gpsimd.tensor_copy(
                                at8[:, hh2, 0, q0:q0 + CH], at_t[:, :])
                            nc.gpsimd.tensor_tensor(
                                at8[:, hh2, 1, q0:q0 + CH], at_t[:, :],
                                at8[:, hh2, 0, q0:q0 + CH],
                                mybir.AluOpType.subtract)
                        if filler is not None:
                            for fst, a, b in filler[h]:
                                emit_p3_st(fst, a, b)

                def emit_p3_st(st, oc0=0, oc1=H // CH, act_frac=4):
                    """Output projection for s-tile st (oc strips [oc0,oc1)):
                    compensated fp8 DoubleRow over the 4 local heads (2 hi@hi
                    pair insts + 4 cross insts per psum tile). PSUM comes
                    from the proj pool (idle whenever this runs). act_frac:
                    1-in-N copies go to ACT (use 2 when no exp pressure,
                    4 when interleaved with attention)."""
                    ssl = bass.ds(st * 128, 128)
                    # the very last tile ends with two per-512 strips so the
                    # final drain is one small DMA after a short copy
                    for oc in range(oc0, oc1):
                        osl = bass.ds(oc * CH, CH)
                        strip = 1 if (st == NKT - 1 and oc >= 6) else 2
                        o0 = oc - (oc % 2 if strip == 2 else 0)
                        if oc == o0:
                            ob = out_pool.tile(
                                [128, strip * CH], BF, tag=f"ob{strip}",
                                name=f"ob_{_rep}_{st}_{oc}")
                        pw = proj_pool.tile([128, CH], F32, tag="pp",
                                            name=f"pw_{_rep}_{st}_{oc}")
                        for j in range(2):
                            nc.tensor.matmul(
                                pw[:, :],
                                (at8a if j == 0 else at8b)[:, :, 0, ssl],
                                wo_sb[:, 2 * j:2 * j + 2, 1, osl],
                                start=(j == 0), stop=False, perf_mode=DR)
                        for hh in range(NHL):
                            nc.tensor.matmul(
                                pw[:, :],
                                (at8a if hh < 2 else at8b)[:, hh % 2, :, ssl],
                                wo_sb[:, hh, :, osl],
                                start=False, stop=(hh == NHL - 1),
                                perf_mode=DR)
                        # alternate copy engine to split PSUM->SBUF load;
                        # descale the fp8 weight prescale
                        half = bass.ds((oc - o0) * CH, CH)
                        if act_frac == 1 or (st * (H // CH) + oc) \
                                % act_frac == act_frac - 1:
                            nc.scalar.mul(ob[:, half], pw[:, :], DESC)
                        else:
                            nc.vector.tensor_scalar_mul(
                                ob[:, half], pw[:, :], DESC)
                        if oc == o0 + strip - 1:
                            # strips emitted before the last hst chunk go on
                            # the ACT queue: on SP their wait for the ob copy
                            # would head-block chunk-3's hst stream
                            dma = (nc.scalar.dma_start
                                   if (st < 4 or (st == NKT - 1 and oc % 2))
                                   else nc.sync.dma_start)
                            dma(out=outp[st * 128:(st + 1) * 128,
                                         o0 * CH:(o0 + strip) * CH],
                                in_=ob[:, :])

                def emit_p3_staggered(st):
                    """First tile after a chunk's attention: the at8b pair
                    still waits on the last head's hi/lo split, so emit the
                    at8a-dependent halves of three psum groups first."""
                    ssl = bass.ds(st * 128, 128)
                    pws = []
                    # ps/po slots are free the moment the chunk's attention
                    # ends; proj slots wait on the tail filler's copies, so
                    # they take the later groups
                    lend = [(ps_pool, "ps"), (po_pool, "po"),
                            (proj_pool, "pp"), (proj_pool, "pp")]
                    for oc in range(4):
                        osl = bass.ds(oc * CH, CH)
                        pw = lend[oc][0].tile([128, CH], F32, tag=lend[oc][1],
                                              name=f"pw_{_rep}_{st}_{oc}")
                        nc.tensor.matmul(pw[:, :], at8a[:, :, 0, ssl],
                                         wo_sb[:, 0:2, 1, osl],
                                         start=True, stop=False, perf_mode=DR)
                        for hh in range(2):
                            nc.tensor.matmul(pw[:, :],
                                             at8a[:, hh, :, ssl],
                                             wo_sb[:, hh, :, osl],
                                             start=False, stop=False,
                                             perf_mode=DR)
                        pws.append(pw)
                    for oc in range(4):
                        osl = bass.ds(oc * CH, CH)
                        pw = pws[oc]
                        if oc % 2 == 0:
                            ob = out_pool.tile([128, 2 * CH], BF, tag="ob2",
                                               name=f"ob_{_rep}_{st}_{oc}")
                            obs = ob
                        else:
                            ob = obs
                        nc.tensor.matmul(pw[:, :], at8b[:, :, 0, ssl],
                                         wo_sb[:, 2:4, 1, osl],
                                         start=False, stop=False, perf_mode=DR)
                        for hh in range(2):
                            nc.tensor.matmul(pw[:, :],
                                             at8b[:, hh, :, ssl],
                                             wo_sb[:, 2 + hh, :, osl],
                                             start=False, stop=(hh == 1),
                                             perf_mode=DR)
                        half = bass.ds((oc % 2) * CH, CH)
                        if (st * (H // CH) + oc) % 4 == 3:
                            nc.scalar.mul(ob[:, half], pw[:, :], DESC)
                        else:
                            nc.vector.tensor_scalar_mul(
                                ob[:, half], pw[:, :], DESC)
                        if oc % 2 == 1:
                            nc.sync.dma_start(
                                out=outp[st * 128:(st + 1) * 128,
                                         (oc - 1) * CH:(oc + 1) * CH],
                                in_=ob[:, :])
                    emit_p3_st(st, 4, H // CH)

                # Phase order = PE program order. Chunk-0 attention directly
                # follows chunk-0 projections (fills the chunk-1 hst DMA
                # window); later attention chunks interleave one output-
                # projection s-tile per head so the exp-paced stretches keep
                # PE fed; p3 uses the proj PSUM bufs, which are free during
                # every p2/p3 stretch.
                def lagged(s0):
                    """Half-tile-lagged filler: head h gets the back half of
                    tile s0+h-1 and the front half of s0+h, so the last
                    head's DVE/ACT tail is covered by the leftover back half
                    emitted right after the chunk."""
                    oc4 = H // CH // 2
                    fill = [[(s0, 0, oc4)]]
                    for h in range(1, NHL):
                        fill.append([(s0 + h - 1, oc4, 2 * oc4),
                                     (s0 + h, 0, oc4)])
                    return fill

                emit_p1(0, load_weights=(_rep == 0))
                emit_p2(0)
                emit_p1(1)
                if _rep == 0:
                    nc.sync.dma_start(out=wo_sb[:, 0:2], in_=wo_r[:, 0:2])
                    nc.sync.dma_start(out=wo_sb[:, 2:4], in_=wo_r[:, 2:4])
                emit_p1(2)
                emit_p2(1, filler=lagged(0), prefill=0)
                emit_p3_st(3, H // CH // 2, H // CH)
                emit_p1(3)
                emit_p2(2, filler=lagged(4), prefill=4)
                emit_p3_st(7, H // CH // 2, H // CH)
                emit_p2(3, filler=lagged(8), prefill=8)
                emit_p3_st(11, H // CH // 2, H // CH)
                emit_p3_staggered(12)
                for st in range(13, 16):
                    emit_p3_st(st)

    _split_excess_waits(nc)
    return nc


_NC_CACHE = None


def _get_nc():
    global _NC_CACHE
    if _NC_CACHE is None:
        _NC_CACHE = build_bass()
    return _NC_CACHE


def _hilo_rows(x, order):
    """x [K, M] fp32 (pre-scaled) -> [(ktile p two), M] f8 with the residual
    split; order 'hilo' for stationary/lhsT tensors, 'lohi' for moving/rhs."""
    hi = x.astype(f8)
    lo = (x - hi.astype(np.float32)).astype(f8)
    a, b = (hi, lo) if order == "hilo" else (lo, hi)
    kt = x.shape[0] // 128
    out = np.empty((kt, 128, 2, x.shape[1]), dtype=f8)
    out[:, :, 0] = a.reshape(kt, 128, -1)
    out[:, :, 1] = b.reshape(kt, 128, -1)
    return out.reshape(kt * 128 * 2, x.shape[1])


def make_in_maps(hidden_states, cos, sin, Wq, Wk, Wv, Wo, sinks):
    scaling = HD ** -0.5
    hs = np.asarray(hidden_states, dtype=np.float32).reshape(S, H)
    hsT = np.ascontiguousarray(hs.T)
    # hs8: rows (ktile, partition), cols (chunk, two=(lo,hi), s-in-chunk)
    hi = hsT.astype(f8)
    lo = (hsT - hi.astype(np.float32)).astype(f8)
    A = np.empty((HT, 128, NCHUNK, 2, CH), dtype=f8)
    A[:, :, :, 0] = lo.reshape(HT, 128, NCHUNK, CH)
    A[:, :, :, 1] = hi.reshape(HT, 128, NCHUNK, CH)
    hs8 = A.reshape(HT * 128, 2 * S)
    cosT = np.ascontiguousarray(np.asarray(cos, np.float32).reshape(S, ROPE).T)
    sinT = np.ascontiguousarray(np.asarray(sin, np.float32).reshape(S, ROPE).T)
    sinTs = sinT.copy()
    sinTs[:ROPE // 2] *= -1.0
    cosT = (cosT * DESC).astype(bf16)
    sinTs = (sinTs * DESC).astype(bf16)
    Wq = np.asarray(Wq, np.float32)
    Wk = np.asarray(Wk, np.float32)
    Wv = np.asarray(Wv, np.float32)
    Wo = np.asarray(Wo, np.float32)
    sinks = np.asarray(sinks, np.float32)
    maskb = ((np.arange(1024)[None, :] - 512) >= np.arange(128)[:, None])
    maskb = maskb.astype(np.float32).astype(bf16)
    identity = np.eye(128, dtype=np.float32).astype(bf16)

    in_maps = []
    for c in range(N_CORES):
        qcols = slice(NHL * HD * c, NHL * HD * (c + 1))
        kcols = slice(HD * c, HD * (c + 1))
        esink_c = (np.exp(sinks[NHL * c:NHL * (c + 1)]) / 128.0).astype(np.float32)
        in_maps.append({
            "hs8": hs8,
            "wq8": _hilo_rows(Wq[:, qcols] * (scaling * SW), "hilo"),
            "wk8": _hilo_rows(Wk[:, kcols] * SW, "hilo"),
            "wv8": _hilo_rows(Wv[:, kcols] * SW, "hilo"),
            "wo8": _hilo_rows(Wo[qcols, :] * SW, "lohi"),
            "cosT": cosT,
            "sinTs": sinTs,
            "esink": np.repeat(esink_c[None, :], 128, axis=0).copy(),
            "maskb": maskb,
            "ident": identity,
        })
    return in_maps


def kernel(hidden_states, cos, sin, attention_mask, Wq, Wk, Wv, Wo, sinks):
    # attention_mask is the standard causal mask; causality is built into the
    # kernel (binary masks on the diagonal score tiles), so it is unused.
    in_maps = make_in_maps(hidden_states, cos, sin, Wq, Wk, Wv, Wo, sinks)
    nc = _get_nc()
    res = run_bass_kernel_spmd(nc, in_maps, core_ids=list(range(N_CORES)))
    acc = np.zeros((S, H), dtype=np.float32)
    for r in res.results:
        acc += r["outp"].astype(np.float32)
    return acc.reshape(1, S, H)


# revision 96
# speedup vs baseline: 1.0017x; 1.0016x over previous
"""MiMoV2 attention (GQA + partial RoPE + attention sinks + causal) on 8 TRN2
NeuronCores.

Sharding: tensor-parallel over heads. Core c owns KV head c and query heads
[4c, 4c+4). Wq/Wk/Wv split along output dim, Wo along input dim; each core
computes a partial output [S, H] which the host sums (the Wo contraction over
heads distributes over cores).

Per-core dataflow (everything head-transposed so no on-chip transposes needed):
  hsT [H, S] streamed by 128-row h-tiles; per s-chunk of 512:
    QT[d, s] (4 heads), KT[d, s] accumulate in PSUM over the h-tiles
    V[s, d] natural layout via hsT-as-stationary matmuls

  The QKV and output projections run as error-compensated fp8 matmuls in
  DoubleRow perf mode: x ~= x_hi + x_lo (both f8e4m3, residual split), and
  x@w = xh@wh + (xh@wl + xl@wh), dropping the lo*lo term (~0.07% error,
  below bf16 noise). The hi@hi pass packs two 128-deep k-tiles per DoubleRow
  instruction; the cross pass packs both cross products of one k-tile per
  instruction (w stored (hi,lo), x stored (lo,hi)). 3 passes over 2x-rate
  fp8 = 0.75x the PE cycles of bf16 at ~equal accuracy. Weights are
  pre-scaled by 512 on the host (fp8 subnormal floor), descaled in the
  PSUM->SBUF copy-outs (rope cos/sin pre-scaled, ACT copy-with-scale
  elsewhere).

  partial RoPE applied in [d, s] layout; rotate_half reads the PSUM rows
  cross-partition with the sign folded into sinT. V transposes to [s, d]
  via PE transpose matmuls (HWDGE generation is a serial resource the hst
  stream needs). scoresT[s_k, s_q] = KT_tile^T @ QT-chunk (bf16: with a
  128-deep contraction, 3-pass fp8 DoubleRow would cost 1.5x, so bf16 is
  optimal here); exp on ACT (no max subtraction -- |scores| <= ~12);
  causal via binary mask multiply on diagonal tiles only. attn_outT[d,
  s_q] accumulates V_tile^T @ probsT (bf16). The softmax denominator
  accumulates the probs tiles on DVE and Pool (2:1, matching their
  throughputs) seeded with exp(sink)/128, and finishes with two all-ones
  matmuls per (chunk, head) -- vs one per score tile -- freeing ~8% of PE
  cycles; division by DVE reciprocal + multiply, then the attnT chunk is
  split hi/lo fp8 (Pool; DVE for the head gating the final phase).
  out_partial[s, o] = attnT-as-stationary @ Wo-chunk (fp8 DoubleRow over
  the 4 local heads), interleaved tile-by-tile into the exp-paced
  attention chunks; written out as bf16 in 1024-col strips; host sums the
  8 partials in fp32.

  Emission order is the PE schedule: p1(0), p2(0), p1(1), p1(2),
  p2(1)+p3(0), p1(3), p2(2)+p3(1), p2(3)+p3(2), p3(3), with p3 s-tiles
  half-lagged behind the heads so the last head's softmax tail overlaps
  projection matmuls.
"""

import numpy as np
import ml_dtypes
from contextlib import ExitStack

import concourse.bass as bass
import concourse.mybir as mybir
import concourse.tile as tile
from concourse.bass_utils import run_bass_kernel_spmd

bf16 = ml_dtypes.bfloat16
f8 = ml_dtypes.float8_e4m3
BF = mybir.dt.bfloat16
F32 = mybir.dt.float32
F8 = mybir.dt.float8e4
DR = mybir.MatmulPerfMode.DoubleRow

N_CORES = 8
S = 2048
H = 4096
HD = 128
ROPE = 64
NHL = 4                    # local query heads per core
CH = 512                   # s-chunk width
NCHUNK = S // CH           # 4
HT = H // 128              # 32 h-tiles
NKT = S // 128             # 16 k-tiles
SW = 512.0                 # host-side fp8 weight pre-scale (power of 2)
DESC = 1.0 / SW

# this walrus build allows at most one sync wait per instruction
_MAX_WAITS = 1


def _split_excess_waits(nc):
    cnt = 0
    for f in nc.m.functions:
        for bb in f.blocks:
            out, changed = [], False
            for inst in bb.instructions:
                si = inst.sync_info
                if si is not None and len(si.on_wait) > _MAX_WAITS:
                    waits = list(si.on_wait)
                    excess, keep = waits[:-_MAX_WAITS], waits[-_MAX_WAITS:]
                    for i in range(0, len(excess), _MAX_WAITS):
                        cnt += 1
                        out.append(mybir.InstNoOp(
                            name=f"waitnop-{cnt}", engine=inst.engine,
                            sync_info=mybir.SyncInfo(
                                on_wait=excess[i:i + _MAX_WAITS], on_update=[])))
                    si.on_wait = keep
                    changed = True
                out.append(inst)
            if changed:
                bb.instructions = out
    return cnt


def _rope_copy(nc, pool, psum_t, dest, cos_sb, sin_sb, sl):
    """psum_t [128,512] fp32 -> dest [128,512] bf16 slice, applying partial
    RoPE to rows 0:64 (rotate_half = +-32-partition swap done by reading the
    PSUM rows cross-partition, sign pre-folded into sin_sb). cos/sin are
    pre-scaled by DESC on the host, and the pass-through copy descales on
    ACT, so the fp8 weight scale cancels here.
    """
    # pass-through rows 64:128 on ACT (keeps DVE free), descaled
    nc.scalar.mul(dest[64:128, :], psum_t[64:128, :], DESC)
    # t2 = rotate_half(q_r) * sin, reading the swapped halves straight from
    # PSUM (cross-partition operand offsets)
    t2 = pool.tile([64, CH], BF, tag="rope_t2")
    nc.vector.tensor_mul(t2[0:32, :], psum_t[32:64, :], sin_sb[0:32, sl])
    nc.vector.tensor_mul(t2[32:64, :], psum_t[0:32, :], sin_sb[32:64, sl])
    # t1 = q_r * cos   (one fused op: (psum mult 1.0) mult cos)
    t1 = pool.tile([64, CH], BF, tag="rope_t1")
    nc.vector.scalar_tensor_tensor(
        t1[:, :], psum_t[0:64, :], 1.0, cos_sb[:, sl],
        op0=mybir.AluOpType.mult, op1=mybir.AluOpType.mult)
    nc.vector.tensor_add(dest[0:64, :], t1[:, :], t2[:, :])


def build_bass(repeat=1):
    """repeat>1 duplicates the whole compute body (for timing: the wall-clock
    delta between repeat=2 and repeat=1 NEFFs is one kernel iteration,
    independent of the large fixed PJRT/axon dispatch overhead)."""
    nc = bass.Bass("TRN2", target_bir_lowering=False, debug=False)

    # fp8 hi/lo-split tensors: `two` holds (lo, hi) for moving/rhs-side
    # tensors and (hi, lo) for stationary/lhsT-side tensors so one DoubleRow
    # instruction covers both cross terms. hs8 rows are (ktile, partition)
    # and columns (chunk, two, s) so one 3-dim DMA moves a whole
    # [4-ktile, both-plane, chunk] tile.
    hs8 = nc.dram_tensor("hs8", [HT * 128, 2 * S], F8, kind="ExternalInput")
    wq8 = nc.dram_tensor("wq8", [HT * 128 * 2, NHL * HD], F8, kind="ExternalInput")
    wk8 = nc.dram_tensor("wk8", [HT * 128 * 2, HD], F8, kind="ExternalInput")
    wv8 = nc.dram_tensor("wv8", [HT * 128 * 2, HD], F8, kind="ExternalInput")
    wo8 = nc.dram_tensor("wo8", [NHL * 128 * 2, H], F8, kind="ExternalInput")
    cosT = nc.dram_tensor("cosT", [ROPE, S], BF, kind="ExternalInput")
    sinTs = nc.dram_tensor("sinTs", [ROPE, S], BF, kind="ExternalInput")
    esink = nc.dram_tensor("esink", [128, NHL], F32, kind="ExternalInput")
    ident = nc.dram_tensor("ident", [128, 128], BF, kind="ExternalInput")
    maskb = nc.dram_tensor("maskb", [128, 1024], BF, kind="ExternalInput")
    outp = nc.dram_tensor("outp", [S, H], BF, kind="ExternalOutput")

    with tile.TileContext(nc) as tc, ExitStack() as ctx:
        const = ctx.enter_context(tc.tile_pool(name="const", bufs=1))
        hs_pool = ctx.enter_context(tc.tile_pool(name="hs", bufs=9))
        rope_pool = ctx.enter_context(tc.tile_pool(name="rope", bufs=2))
        probs_pool = ctx.enter_context(tc.tile_pool(name="probs", bufs=9))
        den_pool = ctx.enter_context(tc.tile_pool(name="den", bufs=2))
        dacc_pool = ctx.enter_context(tc.tile_pool(name="dacc", bufs=2))
        att_pool = ctx.enter_context(tc.tile_pool(name="att", bufs=3))
        out_pool = ctx.enter_context(tc.tile_pool(name="out", bufs=4))

        # ---- constants / weights resident in SBUF ----
        # weights are loaded in h-tile groups so the first projection matmuls
        # only wait on the first slice, not the whole tensor
        wq_sb = const.tile([128, HT, 2, NHL * HD], F8)
        wk_sb = const.tile([128, HT, 2, HD], F8)
        wv_sb = const.tile([128, HT, 2, HD], F8)
        wq_r = wq8.rearrange("(t p two) c -> p t two c", p=128, two=2)
        wk_r = wk8.rearrange("(t p two) c -> p t two c", p=128, two=2)
        wv_r = wv8.rearrange("(t p two) c -> p t two c", p=128, two=2)
        hsT_r = hs8.rearrange("(t p) (c x) -> p t c x", p=128, c=NCHUNK)
        wo_sb = const.tile([128, NHL, 2, H], F8)
        wo_r = wo8.rearrange("(t p two) c -> p t two c", p=128, two=2)
        cos_sb = const.tile([ROPE, S], BF)
        sin_sb = const.tile([ROPE, S], BF)
        mask_sb = const.tile([128, 1024], BF)
        esink_sb = const.tile([128, NHL], F32)
        nc.gpsimd.dma_start(out=esink_sb, in_=esink[:, :])
        ident_sb = const.tile([128, 128], BF)
        nc.gpsimd.dma_start(out=ident_sb, in_=ident[:, :])
        ones_sb = const.tile([128, 128], BF)
        nc.vector.memset(ones_sb[:, :], 1.0)

        # persistent activations
        qt_sb = const.tile([128, NHL, S], BF)     # QT per head [d, s]
        kt_sb = const.tile([128, S], BF)          # KT [d, s]
        vt_sb = const.tile([128, S], BF)          # VT [d, s] (pre-transpose)
        v_sb = const.tile([128, NKT, HD], BF)     # V [s(128), kt, d]
        # attnT (hi,lo) [d, s], split by head PAIR: coarse per-tile write
        # tracking otherwise makes every p3 matmul wait on the latest head's
        # split (the hi@hi insts need each pair adjacent, so no finer split)
        at8a = const.tile([128, 2, 2, S], F8)
        at8b = const.tile([128, 2, 2, S], F8)

        for _rep in range(repeat):
            # phases 1+2 share one PSUM scope (8 banks: proj 3 + ps 2 + po 2
            # + pd 1) so projection chunks and attention chunks interleave on
            # PE with no pool-boundary serialization.
            with ExitStack() as p12:
                proj_pool = p12.enter_context(
                    tc.tile_pool(name="proj", bufs=3, space="PSUM"))
                ps_pool = p12.enter_context(
                    tc.tile_pool(name="ps", bufs=2, space="PSUM"))
                po_pool = p12.enter_context(
                    tc.tile_pool(name="po", bufs=2, space="PSUM"))
                ptp_pool = p12.enter_context(
                    tc.tile_pool(name="ptp", bufs=1, space="PSUM"))

                def emit_p1(ci, load_weights=False):
                    """QKV projections + RoPE for s-chunk ci. Each group's 48
                    DoubleRow matmuls (16 hi@hi ktile-pairs + 32 cross) form
                    one contiguous PSUM accumulation group in a single bank."""
                    sl = bass.ds(ci * CH, CH)
                    hs4 = []
                    for g4 in range(HT // 4):
                        h4 = hs_pool.tile([128, 4, 2, CH], F8, tag="hst",
                                          name=f"hst_{_rep}_{ci}_{g4}")
                        g = g4 * 4
                        # DMA APs allow max 3 dims: the s-chunk slice blocks
                        # (ktile,two) merging, so move each fp8 plane
                        # separately. The hst stream owns the SP queue; the
                        # weights go on the Pool queue so neither blocks the
                        # other at the queue head.
                        if load_weights and g4 % 2 == 0:
                            # interleave weight-slice loads with the hst
                            # stream (k first: the first matmuls are group
                            # k's) so the first matmuls start early
                            nc.sync.dma_start(out=wk_sb[:, g:g + 8],
                                              in_=wk_r[:, g:g + 8])
                        nc.sync.dma_start(out=h4, in_=hsT_r[:, g:g + 4, ci, :])
                        if load_weights and g4 in (0, 1, 6):
                            if g4 == 6:
                                # the tail of the stream is latency-critical:
                                # split the last wq batch so block 6 doesn't
                                # wait on the full 1MB transfer
                                nc.sync.dma_start(out=wq_sb[:, 24:28],
                                                  in_=wq_r[:, 24:28])
                                nc.sync.dma_start(out=wq_sb[:, 28:32],
                                                  in_=wq_r[:, 28:32])
                            elif g4 == 0:
                                # head of the stream likewise: q0's first
                                # block waits on wq, and hst-g1 queues behind
                                nc.sync.dma_start(out=wq_sb[:, 0:4],
                                                  in_=wq_r[:, 0:4])
                            else:
                                nc.sync.dma_start(out=wq_sb[:, 4:8],
                                                  in_=wq_r[:, 4:8])
                        elif load_weights and g4 % 2 == 0:
                            nc.sync.dma_start(out=wq_sb[:, g:g + 8],
                                              in_=wq_r[:, g:g + 8])
                        if load_weights:
                            if g4 == 5:
                                # rope/mask constants: queued behind the
                                # critical weight stream, ready well
                                # before the first rope copy-out
                                nc.sync.dma_start(out=cos_sb,
                                                  in_=cosT[:, :])
                                nc.sync.dma_start(out=sin_sb,
                                                  in_=sinTs[:, :])
                                nc.sync.dma_start(out=mask_sb,
                                                  in_=maskb[:, :])
                        hs4.append(h4)
                    if load_weights:
                        # wv rides at the end of the stream: the v group's
                        # matmuls are deferred past the interleave, so its
                        # 1MB stays off the critical early window
                        nc.sync.dma_start(out=wv_sb[:, 0:16],
                                          in_=wv_r[:, 0:16])
                        nc.sync.dma_start(out=wv_sb[:, 16:32],
                                          in_=wv_r[:, 16:32])

                    def copy_out(pp, rope):
                        if rope is not None:
                            _rope_copy(nc, rope_pool, pp, rope, cos_sb, sin_sb, sl)
                        else:
                            nc.vector.tensor_scalar_mul(vt_sb[:, sl], pp[:, :],
                                                        DESC)
                            # transpose to [s, d] on PE (53ns/tile) instead of
                            # 4 DMAs: HWDGE generation is a serial resource
                            # the hst stream needs
                            ptp = ptp_pool.tile([128, 4 * HD], BF, tag="ptp",
                                                 name=f"ptp_{_rep}_{ci}")
                            for st in range(4):
                                kj = ci * 4 + st
                                nc.tensor.matmul(
                                    ptp[:, st * HD:(st + 1) * HD],
                                    vt_sb[:, kj * 128:(kj + 1) * 128],
                                    ident_sb[:, :],
                                    start=(st == 0), stop=(st == 3),
                                    is_transpose=True)
                            nc.scalar.copy(v_sb[:, ci * 4:(ci + 1) * 4, :],
                                           ptp[:, :])

                    # groups: (weight tile, cols, copy-out dest, name); k and
                    # q0 lead so head 0's attention inputs finish first, v
                    # third so vt is written before the q1-3 rope chain
                    groups = [
                        (wk_sb, bass.ds(0, HD), kt_sb[:, sl], "k"),
                        (wq_sb, bass.ds(0, HD), qt_sb[:, 0, sl], "q0"),
                        (wv_sb, bass.ds(0, HD), None, "v"),
                    ] + [
                        (wq_sb, bass.ds(h * HD, HD), qt_sb[:, h, sl], f"q{h}")
                        for h in range(1, NHL)
                    ]

                    def emit_block(gi, b, pp):
                        """6 DoubleRow matmuls covering ktiles 4b..4b+4 of
                        group gi: 2 hi@hi pair insts + 4 cross insts."""
                        w_sb, cols, _, _ = groups[gi]
                        first = (b == 0)
                        last = (b == HT // 4 - 1)
                        for j in range(2):          # ktile pairs
                            pt = 2 * b + j
                            t0 = 2 * pt
                            nc.tensor.matmul(
                                pp[:, :],
                                w_sb[:, t0:t0 + 2, 0, cols],
                                hs4[t0 // 4][:, (t0 % 4):(t0 % 4) + 2, 1, :],
                                start=(first and j == 0), stop=False,
                                perf_mode=DR)
                        for j in range(4):          # cross terms per ktile
                            t = 4 * b + j
                            nc.tensor.matmul(
                                pp[:, :],
                                w_sb[:, t, :, cols],
                                hs4[t // 4][:, t % 4, :, :],
                                start=False, stop=(last and j == 3),
                                perf_mode=DR)

                    if load_weights:
                        # chunk 0 is paced by the input DMA stream: interleave
                        # ALL 6 groups across arriving hst tiles so PE keeps
                        # up with the DMA rate. The attention PSUM pools are
                        # idle during chunk 0, so borrow their banks (every
                        # tile here is the same [128, CH] fp32 = 1 bank).
                        lenders = [(proj_pool, "pp"), (proj_pool, "pp"),
                                   (proj_pool, "pp"), (ps_pool, "ps"),
                                   (po_pool, "po"), (ps_pool, "ps")]
                        # k,v,q0 on proj; q1->ps, q2->po, q3->pd: the borrowed
                        # banks release in time for p2(0)'s first tiles
                        pps = [pool.tile([128, CH], F32, tag=tg,
                                         name=f"pp_{_rep}_{ci}_{g[3]}")
                               for (pool, tg), g in zip(lenders, groups)]
                        # interleaved while the DMA stream is the limiter
                        # (v deferred: its weights arrive last)...
                        B0 = 6
                        for b in range(B0):
                            for gi in (0, 1, 3, 4, 5):
                                emit_block(gi, b, pps[gi])
                        # ...then staggered tails so each group's copy-out
                        # chain overlaps the next group's matmuls; v's full
                        # block run goes third, against the stream's tail
                        for gi in (0, 1, 2, 3, 4, 5):
                            b0 = 0 if gi == 2 else B0
                            for b in range(b0, HT // 4):
                                emit_block(gi, b, pps[gi])
                            copy_out(pps[gi], groups[gi][2])
                    else:
                        for gi in range(6):
                            pp = proj_pool.tile(
                                [128, CH], F32, tag="pp",
                                name=f"pp_{_rep}_{ci}_{groups[gi][3]}")
                            for b in range(HT // 4):
                                emit_block(gi, b, pp)
                            copy_out(pp, groups[gi][2])

                def emit_p2(ci, filler=None, prefill=None):
                    """Attention for query chunk ci, all 4 local heads.
                    Emission is software-pipelined: scores(kj+1) is emitted
                    before attnV(kj) so PE computes the next score tile while
                    ACT does exp of the previous one. `filler` optionally
                    names one earlier-chunk output-projection s-tile to emit
                    after each head: attention alone is exp(ACT)-paced, so
                    the interleaved projection matmuls keep PE fed."""
                    q0 = ci * CH
                    n_kt = 4 * (ci + 1)
                    for h in range(NHL):
                        po = po_pool.tile([128, CH], F32,
                                          name=f"po_{_rep}_{ci}_{h}", tag="po")
                        # two denominator accumulators: even score tiles sum
                        # on DVE, odd on the (otherwise idle) Pool engine
                        dacc = dacc_pool.tile([128, CH], BF, tag="dacc",
                                              name=f"dacc_{_rep}_{ci}_{h}")
                        if not (ci == NCHUNK - 1 and h == NHL - 1):
                            dacp = dacc_pool.tile([128, CH], BF, tag="dacp",
                                                  name=f"dacp_{_rep}_{ci}_{h}")
                        stage = []  # (kj, ps, pr, off)
                        last_pr = []

                        def emit_scores(kj):
                            off = kj * 128 - q0
                            ps = ps_pool.tile([128, CH], F32,
                                              name=f"ps_{_rep}_{ci}_{h}_{kj}",
                                              tag="ps")
                            kt_t = kt_sb[:, kj * 128:(kj + 1) * 128]
                            if off > 0:
                                # columns < off are fully masked: skip them
                                nc.tensor.matmul(ps[:, off:],
                                                 kt_t, qt_sb[:, h, q0 + off:q0 + CH],
                                                 start=True, stop=True)
                            else:
                                nc.tensor.matmul(ps[:, :], kt_t,
                                                 qt_sb[:, h, q0:q0 + CH],
                                                 start=True, stop=True)
                            pr = probs_pool.tile([128, CH], BF,
                                                 name=f"pr_{_rep}_{ci}_{h}_{kj}",
                                                 tag="pr")
                            if off > 0:
                                nc.scalar.activation(
                                    pr[:, off:], ps[:, off:],
                                    mybir.ActivationFunctionType.Exp)
                            else:
                                nc.scalar.activation(
                                    pr[:, :], ps[:, :],
                                    mybir.ActivationFunctionType.Exp)
                            if off >= 0:
                                # triangular 128-col band at q_local in
                                # [off, off+128): maskb[:, 512:640] is the
                                # aligned triangle for every diagonal tile
                                nc.vector.tensor_mul(
                                    pr[:, off:off + 128], pr[:, off:off + 128],
                                    mask_sb[:, 512:640])
                            # denominator: accumulate probs tiles (DVE for
                            # even kj, Pool for odd) so the partition-sum
                            # needs only ONE matmul per (ci,h)
                            lo = max(off, 0)
                            dve_only = (ci == NCHUNK - 1 and h == NHL - 1)
                            if kj == 0:
                                # seed with exp(sink)/128: the all-ones
                                # partition-sum scales it back to exp(sink),
                                # so no separate denominator+sink add is
                                # needed
                                nc.vector.tensor_scalar_add(
                                    dacc[:, :], pr[:, :], esink_sb[:, h:h + 1])
                            elif dve_only:
                                if kj < n_kt - 1:
                                    nc.vector.tensor_add(dacc[:, lo:],
                                                         dacc[:, lo:],
                                                         pr[:, lo:])
                                else:
                                    last_pr[:] = [pr, lo]
                            elif kj == 2:
                                # pr[:, :off] is never written (fully masked
                                # region): seed only the valid columns
                                if lo > 0:
                                    nc.gpsimd.memset(dacp[:, 0:lo], 0.0)
                                    nc.gpsimd.tensor_copy(dacp[:, lo:],
                                                          pr[:, lo:])
                                else:
                                    nc.gpsimd.tensor_copy(dacp[:, :],
                                                          pr[:, :])
                            elif kj % 3 == 2:
                                # 1-in-3 on Pool: its ops cost ~2.1x DVE's
                                nc.gpsimd.tensor_add(dacp[:, lo:],
                                                     dacp[:, lo:], pr[:, lo:])
                            else:
                                nc.vector.tensor_add(dacc[:, lo:],
                                                     dacc[:, lo:], pr[:, lo:])
                            stage.append((kj, ps, pr, off))

                        def emit_av():
                            kj, ps, pr, off = stage.pop(0)
                            fl = dict(start=(kj == 0), stop=(kj == n_kt - 1))
                            if off > 0:
                                nc.tensor.matmul(po[:, off:], v_sb[:, kj, :],
                                                 pr[:, off:], **fl)
                            else:
                                nc.tensor.matmul(po[:, :], v_sb[:, kj, :],
                                                 pr[:, :], **fl)

                        emit_scores(0)
                        for kj in range(1, n_kt):
                            emit_scores(kj)
                            emit_av()
                        emit_av()

                        # partition-sum both accumulators straight into one
                        # PSUM group: a second 512-cycle matmul is cheaper
                        # than a cross-engine merge on the pd critical path
                        pd = ps_pool.tile([128, CH], F32, tag="ps",
                                          name=f"pd_{_rep}_{ci}_{h}")
                        last_head = (ci == NCHUNK - 1 and h == NHL - 1)
                        nc.tensor.matmul(pd[:, :], ones_sb[:, :], dacc[:, :],
                                         start=True, stop=False)
                        if last_head:
                            # the newest tile skips the accumulator: one hop
                            # less on the p3(3) critical chain
                            pr15, lo15 = last_pr[0], last_pr[1]
                            nc.tensor.matmul(pd[:, lo15:], ones_sb[:, :],
                                             pr15[:, lo15:],
                                             start=False, stop=True)
                        else:
                            nc.tensor.matmul(pd[:, :], ones_sb[:, :],
                                             dacp[:, :],
                                             start=False, stop=True)
                        rec = den_pool.tile([128, CH], F32, tag="rec",
                                            name=f"rec_{_rep}_{ci}_{h}")
                        nc.vector.reciprocal(rec[:, :], pd[:, :])
                        at_t = att_pool.tile([128, CH], BF, tag="att",
                                             name=f"att_{_rep}_{ci}_{h}")
                        nc.vector.tensor_mul(at_t[:, :], po[:, :], rec[:, :])
                        # hi/lo fp8 split for the DoubleRow output projection
                        # on Pool: keeps the busy DVE off this non-critical
                        # tail (the split is only needed by the next chunk's
                        # fillers)
                        at8 = at8a if h < 2 else at8b
                        hh2 = h % 2
                        if h == NHL - 1:
                            # last head: DVE (faster) -- its split gates the
                            # next phase's first output-projection tiles
                            nc.vector.tensor_copy(
                                at8[:, hh2, 0, q0:q0 + CH], at_t[:, :])
                            nc.vector.tensor_tensor(
                                at8[:, hh2, 1, q0:q0 + CH], at_t[:, :],
                                at8[:, hh2, 0, q0:q0 + CH],
                                mybir.AluOpType.subtract)
                        else:
                            nc.gpsimd.tensor_copy(
                                at8[:, hh2, 0, q0:q0 + CH], at_t[:, :])
                            nc.gpsimd.tensor_tensor(
                                at8[:, hh2, 1, q0:q0 + CH], at_t[:, :],
                                at8[:, hh2, 0, q0:q0 + CH],
                                mybir.AluOpType.subtract)
                        if filler is not None:
                            for fst, a, b in filler[h]:
                                emit_p3_st(fst, a, b)

                def emit_p3_st(st, oc0=0, oc1=H // CH, act_frac=4):
                    """Output projection for s-tile st (oc strips [oc0,oc1)):
                    compensated fp8 DoubleRow over the 4 local heads (2 hi@hi
                    pair insts + 4 cross insts per psum tile). PSUM comes
                    from the proj pool (idle whenever this runs). act_frac:
                    1-in-N copies go to ACT (use 2 when no exp pressure,
                    4 when interleaved with attention)."""
                    ssl = bass.ds(st * 128, 128)
                    # the very last tile ends with two per-512 strips so the
                    # final drain is one small DMA after a short copy
                    for oc in range(oc0, oc1):
                        osl = bass.ds(oc * CH, CH)
                        strip = 1 if (st == NKT - 1 and oc >= 6) else 2
                        o0 = oc - (oc % 2 if strip == 2 else 0)
                        if oc == o0:
                            ob = out_pool.tile(
                                [128, strip * CH], BF, tag=f"ob{strip}",
                                name=f"ob_{_rep}_{st}_{oc}")
                        pw = proj_pool.tile([128, CH], F32, tag="pp",
                                            name=f"pw_{_rep}_{st}_{oc}")
                        for j in range(2):
                            nc.tensor.matmul(
                                pw[:, :],
                                (at8a if j == 0 else at8b)[:, :, 0, ssl],
                                wo_sb[:, 2 * j:2 * j + 2, 1, osl],
                                start=(j == 0), stop=False, perf_mode=DR)
                        for hh in range(NHL):
                            nc.tensor.matmul(
                                pw[:, :],
                                (at8a if hh < 2 else at8b)[:, hh % 2, :, ssl],
                                wo_sb[:, hh, :, osl],
                                start=False, stop=(hh == NHL - 1),
                                perf_mode=DR)
                        # alternate copy engine to split PSUM->SBUF load;
                        # descale the fp8 weight prescale
                        half = bass.ds((oc - o0) * CH, CH)
                        if act_frac == 1 or (st * (H // CH) + oc) \
                                % act_frac == act_frac - 1:
                            nc.scalar.mul(ob[:, half], pw[:, :], DESC)
                        else:
                            nc.vector.tensor_scalar_mul(
                                ob[:, half], pw[:, :], DESC)
                        if oc == o0 + strip - 1:
                            # strips emitted before the last hst chunk go on
                            # the ACT queue: on SP their wait for the ob copy
                            # would head-block chunk-3's hst stream
                            dma = (nc.scalar.dma_start
                                   if (st < 4 or (st == NKT - 1
                                                  and oc % 2 == 0))
                                   else nc.sync.dma_start)
                            dma(out=outp[st * 128:(st + 1) * 128,
                                         o0 * CH:(o0 + strip) * CH],
                                in_=ob[:, :])

                def emit_p3_staggered(st):
                    """First tile after a chunk's attention: the at8b pair
                    still waits on the last head's hi/lo split, so emit the
                    at8a-dependent halves of three psum groups first."""
                    ssl = bass.ds(st * 128, 128)
                    pws = []
                    # ps/po slots are free the moment the chunk's attention
                    # ends; proj slots wait on the tail filler's copies, so
                    # they take the later groups
                    lend = [(ps_pool, "ps"), (po_pool, "po"),
                            (proj_pool, "pp"), (proj_pool, "pp")]
                    for oc in range(4):
                        osl = bass.ds(oc * CH, CH)
                        pw = lend[oc][0].tile([128, CH], F32, tag=lend[oc][1],
                                              name=f"pw_{_rep}_{st}_{oc}")
                        nc.tensor.matmul(pw[:, :], at8a[:, :, 0, ssl],
                                         wo_sb[:, 0:2, 1, osl],
                                         start=True, stop=False, perf_mode=DR)
                        for hh in range(2):
                            nc.tensor.matmul(pw[:, :],
                                             at8a[:, hh, :, ssl],
                                             wo_sb[:, hh, :, osl],
                                             start=False, stop=False,
                                             perf_mode=DR)
                        pws.append(pw)
                    for oc in range(4):
                        osl = bass.ds(oc * CH, CH)
                        pw = pws[oc]
                        if oc % 2 == 0:
                            ob = out_pool.tile([128, 2 * CH], BF, tag="ob2",
                                               name=f"ob_{_rep}_{st}_{oc}")
                            obs = ob
                        else:
                            ob = obs
                        nc.tensor.matmul(pw[:, :], at8b[:, :, 0, ssl],
                                         wo_sb[:, 2:4, 1, osl],
                                         start=False, stop=False, perf_mode=DR)
                        for hh in range(2):
                            nc.tensor.matmul(pw[:, :],
                                             at8b[:, hh, :, ssl],
                                             wo_sb[:, 2 + hh, :, osl],
                                             start=False, stop=(hh == 1),
                                             perf_mode=DR)
                        half = bass.ds((oc % 2) * CH, CH)
                        if (st * (H // CH) + oc) % 4 == 3:
                            nc.scalar.mul(ob[:, half], pw[:, :], DESC)
                        else:
                            nc.vector.tensor_scalar_mul(
                                ob[:, half], pw[:, :], DESC)
                        if oc % 2 == 1:
                            nc.sync.dma_start(
                                out=outp[st * 128:(st + 1) * 128,
                                         (oc - 1) * CH:(oc + 1) * CH],
                                in_=ob[:, :])
                    emit_p3_st(st, 4, H // CH)

                # Phase order = PE program order. Chunk-0 attention directly
                # follows chunk-0 projections (fills the chunk-1 hst DMA
                # window); later attention chunks interleave one output-
                # projection s-tile per head so the exp-paced stretches keep
                # PE fed; p3 uses the proj PSUM bufs, which are free during
                # every p2/p3 stretch.
                def lagged(s0):
                    """Half-tile-lagged filler: head h gets the back half of
                    tile s0+h-1 and the front half of s0+h, so the last
                    head's DVE/ACT tail is covered by the leftover back half
                    emitted right after the chunk."""
                    oc4 = H // CH // 2
                    fill = [[(s0, 0, oc4)]]
                    for h in range(1, NHL):
                        fill.append([(s0 + h - 1, oc4, 2 * oc4),
                                     (s0 + h, 0, oc4)])
                    return fill

                emit_p1(0, load_weights=(_rep == 0))
                emit_p2(0)
                emit_p1(1)
                if _rep == 0:
                    nc.sync.dma_start(out=wo_sb[:, 0:2], in_=wo_r[:, 0:2])
                    nc.sync.dma_start(out=wo_sb[:, 2:4], in_=wo_r[:, 2:4])
                emit_p1(2)
                emit_p2(1, filler=lagged(0), prefill=0)
                emit_p3_st(3, H // CH // 2, H // CH)
                emit_p1(3)
                emit_p2(2, filler=lagged(4), prefill=4)
                emit_p3_st(7, H // CH // 2, H // CH)
                emit_p2(3, filler=lagged(8), prefill=8)
                emit_p3_st(11, H // CH // 2, H // CH)
                emit_p3_staggered(12)
                for st in range(13, 16):
                    emit_p3_st(st)

    _split_excess_waits(nc)
    return nc


_NC_CACHE = None


def _get_nc():
    global _NC_CACHE
    if _NC_CACHE is None:
        _NC_CACHE = build_bass()
    return _NC_CACHE


def _hilo_rows(x, order):
    """x [K, M] fp32 (pre-scaled) -> [(ktile p two), M] f8 with the residual
    split; order 'hilo' for stationary/lhsT tensors, 'lohi' for moving/rhs."""
    hi = x.astype(f8)
    lo = (x - hi.astype(np.float32)).astype(f8)
    a, b = (hi, lo) if order == "hilo" else (lo, hi)
    kt = x.shape[0] // 128
    out = np.empty((kt, 128, 2, x.shape[1]), dtype=f8)
    out[:, :, 0] = a.reshape(kt, 128, -1)
    out[:, :, 1] = b.reshape(kt, 128, -1)
    return out.reshape(kt * 128 * 2, x.shape[1])


def make_in_maps(hidden_states, cos, sin, Wq, Wk, Wv, Wo, sinks):
    scaling = HD ** -0.5
    hs = np.asarray(hidden_states, dtype=np.float32).reshape(S, H)
    hsT = np.ascontiguousarray(hs.T)
    # hs8: rows (ktile, partition), cols (chunk, two=(lo,hi), s-in-chunk)
    hi = hsT.astype(f8)
    lo = (hsT - hi.astype(np.float32)).astype(f8)
    A = np.empty((HT, 128, NCHUNK, 2, CH), dtype=f8)
    A[:, :, :, 0] = lo.reshape(HT, 128, NCHUNK, CH)
    A[:, :, :, 1] = hi.reshape(HT, 128, NCHUNK, CH)
    hs8 = A.reshape(HT * 128, 2 * S)
    cosT = np.ascontiguousarray(np.asarray(cos, np.float32).reshape(S, ROPE).T)
    sinT = np.ascontiguousarray(np.asarray(sin, np.float32).reshape(S, ROPE).T)
    sinTs = sinT.copy()
    sinTs[:ROPE // 2] *= -1.0
    cosT = (cosT * DESC).astype(bf16)
    sinTs = (sinTs * DESC).astype(bf16)
    Wq = np.asarray(Wq, np.float32)
    Wk = np.asarray(Wk, np.float32)
    Wv = np.asarray(Wv, np.float32)
    Wo = np.asarray(Wo, np.float32)
    sinks = np.asarray(sinks, np.float32)
    maskb = ((np.arange(1024)[None, :] - 512) >= np.arange(128)[:, None])
    maskb = maskb.astype(np.float32).astype(bf16)
    identity = np.eye(128, dtype=np.float32).astype(bf16)

    in_maps = []
    for c in range(N_CORES):
        qcols = slice(NHL * HD * c, NHL * HD * (c + 1))
        kcols = slice(HD * c, HD * (c + 1))
        esink_c = (np.exp(sinks[NHL * c:NHL * (c + 1)]) / 128.0).astype(np.float32)
        in_maps.append({
            "hs8": hs8,
            "wq8": _hilo_rows(Wq[:, qcols] * (scaling * SW), "hilo"),
            "wk8": _hilo_rows(Wk[:, kcols] * SW, "hilo"),
            "wv8": _hilo_rows(Wv[:, kcols] * SW, "hilo"),
            "wo8": _hilo_rows(Wo[qcols, :] * SW, "lohi"),
            "cosT": cosT,
            "sinTs": sinTs,
            "esink": np.repeat(esink_c[None, :], 128, axis=0).copy(),
            "maskb": maskb,
            "ident": identity,
        })
    return in_maps


def kernel(hidden_states, cos, sin, attention_mask, Wq, Wk, Wv, Wo, sinks):
    # attention_mask is the standard causal mask; causality is built into the
    # kernel (binary masks on the diagonal score tiles), so it is unused.
    in_maps = make_in_maps(hidden_states, cos, sin, Wq, Wk, Wv, Wo, sinks)
    nc = _get_nc()
    res = run_bass_kernel_spmd(nc, in_maps, core_ids=list(range(N_CORES)))
    acc = np.zeros((S, H), dtype=np.float32)
    for r in res.results:
        acc += r["outp"].astype(np.float32)
    return acc.reshape(1, S, H)
